# revision 1
# baseline (speedup 1.0000x reference)
"""Dense SE(3) Gauss-Newton kernel for Trainium2, sharded over 8 NeuronCores.

Sharding: core owns batch b = core//4 and a 256-anchor slab of the i axis;
the k axis (1024) runs in 8 chunks of 128 on the partition dimension with
anchors on the free dimension. Per (k,i) quantities reduce over k on the
TensorEngine (float32r single-pass matmuls) against per-k polynomial columns;
the 6x6 normal-equation assembly is itself a matmul against a constant 0/1
combination matrix, then a PE transpose puts anchors back on partitions for
the solve / exp-map / compose stage.
"""
import sys

sys.path.insert(0, "/opt/trn_rl_repo")

import numpy as np

from concourse import bacc, tile, masks
import concourse.mybir as mybir
import concourse.bass_utils as _bu
from concourse.bass_utils import run_bass_kernel_spmd

# Re-enable walrus LDWEIGHTS dedup: consecutive reduction matmuls here share
# one stationary operand, and the per-matmul reload is pure overhead.
if not getattr(_bu, "_ldw_patch", False):
    _orig_run_command = _bu.run_command

    def _run_command_ldw(cmd, *a, **kw):
        cmd = ["--enable-ldw-opt=true" if c == "--enable-ldw-opt=false" else c
               for c in cmd]
        return _orig_run_command(cmd, *a, **kw)

    _bu.run_command = _run_command_ldw
    _bu._ldw_patch = True

F32 = mybir.dt.float32
F32R = mybir.dt.float32r
AF = mybir.ActivationFunctionType
ALU = mybir.AluOpType
AX = mybir.AxisListType

B, C, H, W = 2, 16, 32, 32
N = H * W
NCORES = 8
SLAB = 256
KC = 8
P = 128

# poly columns for the Hm/rhs reduction matmuls
# t:   0  1  2  3  4    5    6    7   8   9   10  11  12  13   14   15   16   17   18
#     [1, x, y, z, x2,  y2,  z2,  xy, xz, yz, -x, -y, -z, -xy, -xz, -yz, 2yz, 2xz, -1]
NT = 19
# acc band order m: 0:M00 1:M11 2:M02p 3:M12p 4:M22 5:g0 6:g1 7:g2n
# band m lives in acc tile m//4 at partitions 0:19, cols (m%4)*SLAB .. +SLAB
NM = 8

# (row, col) of the 6x7 augmented system -> summed acc (m, t) terms
HM_TABLE = {
    (0, 0): [(0, 0)],
    (0, 2): [(2, 18)],
    (0, 3): [(2, 2)],
    (0, 4): [(0, 12), (2, 10)],
    (0, 5): [(0, 2)],
    (0, 6): [(5, 0)],
    (1, 1): [(1, 0)],
    (1, 2): [(3, 18)],
    (1, 3): [(1, 3), (3, 2)],
    (1, 4): [(3, 10)],
    (1, 5): [(1, 10)],
    (1, 6): [(6, 0)],
    (2, 2): [(4, 0)],
    (2, 3): [(3, 12), (4, 11)],
    (2, 4): [(2, 3), (4, 1)],
    (2, 5): [(2, 11), (3, 1)],
    (2, 6): [(7, 18)],
    (3, 3): [(1, 6), (3, 16), (4, 5)],
    (3, 4): [(3, 14), (2, 15), (4, 13)],
    (3, 5): [(1, 14), (2, 5), (3, 13)],
    (3, 6): [(6, 3), (7, 2)],
    (4, 4): [(0, 6), (2, 17), (4, 4)],
    (4, 5): [(0, 15), (2, 13), (3, 4)],
    (4, 6): [(5, 12), (7, 10)],
    (5, 5): [(0, 5), (1, 4)],
    (5, 6): [(5, 2), (6, 10)],
}

NE = 48  # entry columns (6x7 augmented = 42 used, padded)


def combo_matrices():
    """0/1 matrix [NT, NM*NE] mapping acc rows -> augmented-system entries."""
    cm = np.zeros((NT, NM * NE), np.float32)
    for (r, cc), terms in HM_TABLE.items():
        es = [r * 7 + cc]
        if cc < 6 and r != cc:
            es.append(cc * 7 + r)
        for m, t in terms:
            for e in es:
                cm[t, m * NE + e] = 1.0
    return cm


def build_nc():
    nc = bacc.Bacc("TRN2", target_bir_lowering=False, debug=False)

    def din(name, shape):
        return nc.dram_tensor(name, list(shape), F32, kind="ExternalInput")

    # kblob cols: zk 0:8 | uk 8:16 | vk 16:24 | tm_k 24:152 | rev_k 152:176 | w_k 176:200
    kblob_d = din("kblob", (P, 200))
    ek_d = din("ek", (C, N))
    uvzz_d = din("uvzz", (2, N))
    zz2_d = din("zz2", (2, N))
    zo_d = din("zo", (2, N))
    ei_d = din("ei", (C, SLAB))
    cam_d = din("cam", (1, 4))
    cam2_d = din("cam2", (2, 2))
    rhsj_d = din("rhsj", (21, 4 * SLAB))
    cmbt_d = din("cmbt", (NT, NM * NE))
    tmi_d = din("tm_i", (SLAB, 16))
    out_d = nc.dram_tensor("out", [SLAB, 16], F32, kind="ExternalOutput")

    with tile.TileContext(nc) as tc:
        with tc.tile_pool(name="persist", bufs=1) as pp, \
             tc.tile_pool(name="acc_ps", bufs=1, space="PSUM") as accp:

            # ---------------- inputs -> SBUF ----------------
            kblob = pp.tile([P, 200], F32)
            ek = pp.tile([C, N], F32)
            uvzz = pp.tile([2, N], F32)
            zz2 = pp.tile([2, N], F32)
            ei = pp.tile([C, SLAB], F32)
            cam = pp.tile([1, 4], F32)
            zk = kblob[:, 0:8]
            uk = kblob[:, 8:16]
            vk = kblob[:, 16:24]
            tmk = kblob[:, 24:152]
            revk = kblob[:, 152:176]
            wk = kblob[:, 176:200]
            cam2 = pp.tile([2, 2], F32)
            rhsj_s = pp.tile([21, 4 * SLAB], F32)
            cmbt_s = pp.tile([NT, NM * NE], F32)
            rhsj = pp.tile([21, 4 * SLAB], F32R)
            cmbt = pp.tile([NT, NM * NE], F32R)
            tmi0 = pp.tile([P, 16], F32)
            tmi1 = pp.tile([P, 16], F32)
            for t, d in [(kblob, kblob_d), (ek, ek_d), (uvzz, uvzz_d),
                         (zz2, zz2_d), (ei, ei_d), (cam, cam_d),
                         (cam2, cam2_d), (rhsj_s, rhsj_d), (cmbt_s, cmbt_d)]:
                nc.sync.dma_start(t[:], d[:])
            nc.sync.dma_start(tmi0[:], tmi_d[0:P, :])
            nc.sync.dma_start(tmi1[:], tmi_d[P : 2 * P, :])

            ones1 = pp.tile([1, P], F32)
            ones16 = pp.tile([C, 1], F32)
            nc.vector.memset(ones1[:], 1.0)
            nc.vector.memset(ones16[:], 1.0)
            ident = pp.tile([P, P], F32)
            masks.make_identity(nc, ident[:])

            # persistent accumulators: tile j holds bands m=4j..4j+3 as
            # side-by-side column blocks (one start per 2KB psum bank)
            acc_ps0 = accp.tile([32, 4 * SLAB], F32)
            acc_ps1 = accp.tile([32, 4 * SLAB], F32)
            acc_ps = [acc_ps0, acc_ps1]

            # ---------------- setup ----------------
            with tc.tile_pool(name="setup_ps", bufs=1, space="PSUM") as sps:
                cam_psb = sps.tile([P, 4], F32)
                nc.tensor.matmul(cam_psb[:], ones1[:], cam[:], start=True, stop=True)
                camb = pp.tile([P, 4], F32)
                nc.scalar.copy(camb[:], cam_psb[:])

                eisq = pp.tile([C, SLAB], F32)
                nc.scalar.square(eisq[:], ei[:])
                ei2r_ps = sps.tile([1, SLAB], F32)
                nc.tensor.matmul(ei2r_ps[:], ones16[:], eisq[:], start=True, stop=True)
                ei2r = pp.tile([1, SLAB], F32)
                nc.scalar.copy(ei2r[:], ei2r_ps[:])

                eksq = pp.tile([C, N], F32)
                nc.scalar.square(eksq[:], ek[:])
                ek2r_ps = sps.tile([1, N], F32)
                nc.tensor.matmul(ek2r_ps[:, 0 : N // 2], ones16[:],
                                 eksq[:, 0 : N // 2], start=True, stop=True)
                nc.tensor.matmul(ek2r_ps[:, N // 2 : N], ones16[:],
                                 eksq[:, N // 2 : N], start=True, stop=True)
                ek2r = pp.tile([1, N], F32)
                nc.scalar.copy(ek2r[:], ek2r_ps[:])

            nc.sync.dma_start(rhsj_s[19:20, 3 * SLAB : 4 * SLAB], ei2r[:])
            nc.scalar.copy(rhsj[:], rhsj_s[:])
            nc.scalar.copy(cmbt[:], cmbt_s[:])
            rhsjl = pp.tile([21, 4 * SLAB], F32R)
            nc.vector.tensor_tensor(rhsjl[:], rhsj_s[:], rhsj[:].bitcast(F32),
                                    ALU.subtract)

            invfx = pp.tile([P, 1], F32)
            invfy = pp.tile([P, 1], F32)
            nc.vector.reciprocal(invfx[:], camb[:, 0:1])
            nc.vector.reciprocal(invfy[:], camb[:, 2:3])
            negf = pp.tile([P, 2], F32)
            nc.vector.tensor_scalar(negf[:], camb[:, 0:4:2], -1.0, None, ALU.mult)
            f2 = pp.tile([P, 2], F32)  # fx^2, fy^2
            nc.vector.tensor_tensor(f2[:], camb[:, 0:4:2], camb[:, 0:4:2], ALU.mult)
            # weights pre-scaled: w0*fx^2, w1*fy^2 ([128, KC] each)
            wf0 = pp.tile([P, KC], F32)
            wf1 = pp.tile([P, KC], F32)
            nc.vector.tensor_scalar(wf0[:], wk[:, 0 : KC * 3 : 3], f2[:, 0:1], None, ALU.mult)
            nc.vector.tensor_scalar(wf1[:], wk[:, 1 : KC * 3 : 3], f2[:, 1:2], None, ALU.mult)

            # k-major pointcloud x, y
            xk = pp.tile([P, KC], F32)
            yk = pp.tile([P, KC], F32)
            tmpk = pp.tile([P, KC], F32)
            nc.vector.tensor_scalar(tmpk[:], uk[:], camb[:, 1:2], None, ALU.subtract)
            nc.vector.tensor_tensor(tmpk[:], tmpk[:], zk[:], ALU.mult)
            nc.vector.tensor_scalar(xk[:], tmpk[:], invfx[:], None, ALU.mult)
            nc.vector.tensor_scalar(tmpk[:], vk[:], camb[:, 3:4], None, ALU.subtract)
            nc.vector.tensor_tensor(tmpk[:], tmpk[:], zk[:], ALU.mult)
            nc.vector.tensor_scalar(yk[:], tmpk[:], invfy[:], None, ALU.mult)

            # joint stationary [21, N]: rows -2e (16), x, y, z, 1, ek2
            xyzTs = pp.tile([21, N], F32)
            xyzTj = pp.tile([21, N], F32R)
            invf2 = pp.tile([2, 1], F32)
            nc.vector.reciprocal(invf2[:], cam2[:, 1:2])
            stg = pp.tile([2, N], F32)
            nc.vector.tensor_scalar(stg[:], uvzz[:], cam2[:, 0:1], None, ALU.subtract)
            nc.vector.tensor_tensor(stg[:], stg[:], zz2[:], ALU.mult)
            nc.vector.tensor_scalar(stg[:], stg[:], invf2[:], None, ALU.mult)
            nc.scalar.mul(xyzTs[0:C, :], ek[:], -2.0)
            nc.sync.dma_start(xyzTs[C : C + 2, :], stg[:])
            nc.sync.dma_start(xyzTs[C + 2 : C + 4, :], zo_d[:])
            nc.sync.dma_start(xyzTs[C + 4 : C + 5, :], ek2r[:])
            nc.scalar.copy(xyzTj[:], xyzTs[:])
            xyzTl = pp.tile([21, N], F32R)
            nc.vector.tensor_tensor(xyzTl[:], xyzTs[:], xyzTj[:].bitcast(F32),
                                    ALU.subtract)

            # poly columns [128, KC*NT]
            pol_s = pp.tile([P, KC * NT], F32)
            pol = pp.tile([P, KC * NT], F32R)
            E = KC * NT
            def pcol(t):
                return pol_s[:, t:E:NT]
            nc.vector.memset(pcol(0), 1.0)
            nc.vector.memset(pcol(18), -1.0)
            nc.vector.tensor_copy(pcol(1), xk[:])
            nc.vector.tensor_copy(pcol(2), yk[:])
            nc.vector.tensor_copy(pcol(3), zk[:])
            nc.vector.tensor_tensor(pcol(4), xk[:], xk[:], ALU.mult)
            nc.vector.tensor_tensor(pcol(5), yk[:], yk[:], ALU.mult)
            nc.vector.tensor_tensor(pcol(6), zk[:], zk[:], ALU.mult)
            nc.vector.tensor_tensor(pcol(7), xk[:], yk[:], ALU.mult)
            nc.vector.tensor_tensor(pcol(8), xk[:], zk[:], ALU.mult)
            nc.vector.tensor_tensor(pcol(9), yk[:], zk[:], ALU.mult)
            nc.vector.tensor_scalar(pcol(10), xk[:], -1.0, None, ALU.mult)
            nc.vector.tensor_scalar(pcol(11), yk[:], -1.0, None, ALU.mult)
            nc.vector.tensor_scalar(pcol(12), zk[:], -1.0, None, ALU.mult)
            nc.vector.tensor_scalar(pcol(13), pcol(7), -1.0, None, ALU.mult)
            nc.vector.tensor_scalar(pcol(14), pcol(8), -1.0, None, ALU.mult)
            nc.vector.tensor_scalar(pcol(15), pcol(9), -1.0, None, ALU.mult)
            nc.vector.tensor_scalar(pcol(16), pcol(9), 2.0, None, ALU.mult)
            nc.vector.tensor_scalar(pcol(17), pcol(8), 2.0, None, ALU.mult)
            nc.scalar.copy(pol[:], pol_s[:])

            # TjXj rows -> residual bias columns
            def tme(e):
                return tmk[:, e : KC * 16 : 16]
            tjx = [pp.tile([P, KC], F32, name=f"tjx{r}") for r in range(3)]
            sA = pp.tile([P, KC], F32)
            sB = pp.tile([P, KC], F32)
            for r in range(3):
                nc.vector.tensor_tensor(sA[:], tme(4 * r + 0), xk[:], ALU.mult)
                nc.vector.tensor_tensor(sB[:], tme(4 * r + 1), yk[:], ALU.mult)
                nc.vector.tensor_tensor(sA[:], sA[:], sB[:], ALU.add)
                nc.vector.tensor_tensor(sB[:], tme(4 * r + 2), zk[:], ALU.mult)
                nc.vector.tensor_tensor(sA[:], sA[:], sB[:], ALU.add)
                nc.vector.tensor_tensor(tjx[r][:], sA[:], tme(4 * r + 3), ALU.add)
            invzj = pp.tile([P, KC], F32)
            nc.vector.reciprocal(invzj[:], tjx[2][:])
            bias0 = pp.tile([P, KC], F32)
            bias1 = pp.tile([P, KC], F32)
            bias2 = pp.tile([P, KC], F32)
            nc.vector.tensor_tensor(sA[:], tjx[0][:], invzj[:], ALU.mult)
            nc.vector.tensor_scalar(sA[:], sA[:], negf[:, 0:1], None, ALU.mult)
            nc.vector.tensor_tensor(bias0[:], sA[:], revk[:, 0 : KC * 3 : 3], ALU.subtract)
            nc.vector.tensor_tensor(sA[:], tjx[1][:], invzj[:], ALU.mult)
            nc.vector.tensor_scalar(sA[:], sA[:], negf[:, 1:2], None, ALU.mult)
            nc.vector.tensor_tensor(bias1[:], sA[:], revk[:, 1 : KC * 3 : 3], ALU.subtract)
            nc.vector.tensor_scalar(sA[:], invzj[:], -1.0, None, ALU.mult)
            nc.vector.tensor_tensor(bias2[:], sA[:], revk[:, 2 : KC * 3 : 3], ALU.subtract)

            # stores that carry chunk results across the table-batched phases
            dall = pp.tile([P, KC * SLAB], F32)
            XpDall = pp.tile([P, KC * SLAB], F32)
            YpDall = pp.tile([P, KC * SLAB], F32)
            sall = pp.tile([P, KC * SLAB], F32)

            # -------- pipelined passes: A(half) -> batch(half) -> B(half) ----
            d2all = pp.tile([P, KC * SLAB], F32)
            d4all = pp.tile([P, KC * SLAB], F32)
            with tc.tile_pool(name="mm_ps", bufs=2, space="PSUM") as mmp, \
                 tc.tile_pool(name="work", bufs=2) as wp:

                def pass_a(c):
                    ck = slice(c * P, (c + 1) * P)
                    cs = slice(c * SLAB, (c + 1) * SLAB)
                    XY = mmp.tile([P, 2 * SLAB], F32, name=f"XY{c}", tag="XY")
                    ZD = mmp.tile([P, 2 * SLAB], F32, name=f"ZD{c}", tag="ZD")
                    lhs = xyzTj[:, ck]
                    lhsl = xyzTl[:, ck]
                    nc.tensor.matmul(XY[:], lhs, rhsj[:, 0 : 2 * SLAB],
                                     start=True, stop=False)
                    nc.tensor.matmul(XY[:], lhs, rhsjl[:, 0 : 2 * SLAB],
                                     start=False, stop=False)
                    nc.tensor.matmul(XY[:], lhsl, rhsj[:, 0 : 2 * SLAB],
                                     start=False, stop=True)
                    nc.tensor.matmul(ZD[:], lhs, rhsj[:, 2 * SLAB : 4 * SLAB],
                                     start=True, stop=False)
                    nc.tensor.matmul(ZD[:], lhs, rhsjl[:, 2 * SLAB : 4 * SLAB],
                                     start=False, stop=False)
                    nc.tensor.matmul(ZD[:], lhsl, rhsj[:, 2 * SLAB : 4 * SLAB],
                                     start=False, stop=True)
                    d = dall[:, cs]
                    nc.vector.reciprocal(d, ZD[:, 0:SLAB])
                    nc.vector.tensor_tensor(XpDall[:, cs], XY[:, 0:SLAB], d, ALU.mult)
                    nc.vector.tensor_tensor(YpDall[:, cs], XY[:, SLAB : 2 * SLAB], d, ALU.mult)
                    nc.scalar.copy(sall[:, cs], ZD[:, SLAB : 2 * SLAB])

                def batch(h):
                    hs = slice(h * 4 * SLAB, (h + 1) * 4 * SLAB)
                    nc.vector.tensor_scalar(sall[:, hs], sall[:, hs], 0.0, None, ALU.max)
                    nc.scalar.sqrt(sall[:, hs], sall[:, hs])
                    nc.scalar.activation(sall[:, hs], sall[:, hs], AF.Exp, scale=-1.0)
                    nc.vector.tensor_tensor(d2all[:, hs], dall[:, hs], dall[:, hs], ALU.mult)
                    nc.gpsimd.tensor_tensor(d4all[:, hs], d2all[:, hs], d2all[:, hs], ALU.mult)

                def pass_b(c):
                    cs = slice(c * SLAB, (c + 1) * SLAB)
                    d = dall[:, cs]
                    XpD = XpDall[:, cs]
                    YpD = YpDall[:, cs]
                    aff = sall[:, cs]
                    d2 = d2all[:, cs]
                    d4 = d4all[:, cs]
                    afd2 = wp.tile([P, SLAB], F32, name=f"afd2_{c}", tag="afd2")
                    w2pd2 = wp.tile([P, SLAB], F32, name=f"w2pd2_{c}", tag="w2pd2")
                    nc.vector.tensor_tensor(afd2[:], aff, d2, ALU.mult)
                    nc.scalar.mul(w2pd2[:], d2, wk[:, 3 * c + 2 : 3 * c + 3])

                    M00 = wp.tile([P, SLAB], F32R, name=f"M00_{c}", tag="M00")
                    M11 = wp.tile([P, SLAB], F32R, name=f"M11_{c}", tag="M11")
                    M02p = wp.tile([P, SLAB], F32R, name=f"M02p_{c}", tag="M02p")
                    M12p = wp.tile([P, SLAB], F32R, name=f"M12p_{c}", tag="M12p")
                    M22 = wp.tile([P, SLAB], F32R, name=f"M22_{c}", tag="M22")
                    t0 = wp.tile([P, SLAB], F32, name=f"t0_{c}", tag="t0")
                    t1 = wp.tile([P, SLAB], F32, name=f"t1_{c}", tag="t1")
                    t2 = wp.tile([P, SLAB], F32, name=f"t2_{c}", tag="t2")
                    nc.scalar.mul(M00[:], afd2[:], wf0[:, c : c + 1])
                    nc.scalar.mul(M11[:], afd2[:], wf1[:, c : c + 1])
                    nc.vector.tensor_tensor(M02p[:], M00[:].bitcast(F32), XpD, ALU.mult)
                    nc.vector.tensor_tensor(M12p[:], M11[:].bitcast(F32), YpD, ALU.mult)
                    nc.vector.tensor_tensor(t0[:], afd2[:], w2pd2[:], ALU.mult)
                    nc.vector.tensor_tensor(t1[:], M02p[:].bitcast(F32), XpD, ALU.mult)
                    nc.vector.tensor_tensor(t2[:], M12p[:].bitcast(F32), YpD, ALU.mult)
                    nc.vector.tensor_tensor(t1[:], t1[:], t2[:], ALU.add)
                    nc.gpsimd.tensor_tensor(M22[:], t0[:], t1[:], ALU.add)

                    res0 = wp.tile([P, SLAB], F32, name=f"res0_{c}", tag="res0")
                    res1 = wp.tile([P, SLAB], F32, name=f"res1_{c}", tag="res1")
                    res2 = wp.tile([P, SLAB], F32, name=f"res2_{c}", tag="res2")
                    nc.scalar.activation(res0[:], XpD, AF.Identity,
                                         bias=bias0[:, c : c + 1],
                                         scale=camb[:, 0:1])
                    nc.scalar.activation(res1[:], YpD, AF.Identity,
                                         bias=bias1[:, c : c + 1],
                                         scale=camb[:, 2:3])
                    nc.scalar.activation(res2[:], d, AF.Identity,
                                         bias=bias2[:, c : c + 1])

                    g0 = wp.tile([P, SLAB], F32R, name=f"g0_{c}", tag="g0")
                    g1 = wp.tile([P, SLAB], F32R, name=f"g1_{c}", tag="g1")
                    g2n = wp.tile([P, SLAB], F32R, name=f"g2n_{c}", tag="g2n")
                    t3 = wp.tile([P, SLAB], F32, name=f"t3_{c}", tag="t3")
                    t4 = wp.tile([P, SLAB], F32, name=f"t4_{c}", tag="t4")
                    t5 = wp.tile([P, SLAB], F32, name=f"t5_{c}", tag="t5")
                    nc.vector.tensor_tensor(t3[:], d, res0[:], ALU.mult)
                    nc.vector.tensor_tensor(t4[:], d, res1[:], ALU.mult)
                    nc.scalar.mul(g0[:], t3[:], camb[:, 0:1])
                    nc.scalar.mul(g1[:], t4[:], camb[:, 2:3])
                    nc.gpsimd.tensor_tensor(t3[:], XpD, g0[:].bitcast(F32), ALU.mult)
                    nc.gpsimd.tensor_tensor(t4[:], YpD, g1[:].bitcast(F32), ALU.mult)
                    nc.gpsimd.tensor_tensor(t5[:], d2, res2[:], ALU.mult)
                    nc.gpsimd.tensor_tensor(t3[:], t3[:], t4[:], ALU.add)
                    nc.gpsimd.tensor_tensor(g2n[:], t3[:], t5[:], ALU.add)

                    srcs = [M00, M11, M02p, M12p, M22, g0, g1, g2n]
                    polc = pol[:, c * NT : (c + 1) * NT]
                    for m, src in enumerate(srcs):
                        nc.tensor.matmul(
                            acc_ps[m // 4][0:NT, (m % 4) * SLAB : (m % 4 + 1) * SLAB],
                            polc, src[:],
                            start=(c == 0 and m % 2 == 0),
                            stop=(c == KC - 1 and m % 2 == 1))

                for c in range(4):
                    pass_a(c)
                batch(0)
                for c in range(4, KC):
                    pass_a(c)
                for c in range(4):
                    pass_b(c)
                batch(1)
                for c in range(4, KC):
                    pass_b(c)

            # ---------------- entry assembly via matmul -----------------
            with tc.tile_pool(name="post", bufs=2) as qp, \
                 tc.tile_pool(name="post_ps", bufs=2, space="PSUM") as qps:
                acc_sb0 = qp.tile([32, 4 * SLAB], F32R)
                acc_sb1 = qp.tile([32, 4 * SLAB], F32R)
                acc_sb = [acc_sb0, acc_sb1]
                acc_lo0 = qp.tile([32, 4 * SLAB], F32R)
                acc_lo1 = qp.tile([32, 4 * SLAB], F32R)
                acc_lo = [acc_lo0, acc_lo1]
                nc.scalar.copy(acc_sb0[0:NT, :], acc_ps0[0:NT, :])
                nc.scalar.copy(acc_sb1[0:NT, :], acc_ps1[0:NT, :])
                nc.vector.tensor_tensor(acc_lo0[0:NT, :], acc_ps0[0:NT, :],
                                        acc_sb0[0:NT, :].bitcast(F32), ALU.subtract)
                nc.vector.tensor_tensor(acc_lo1[0:NT, :], acc_ps1[0:NT, :],
                                        acc_sb1[0:NT, :].bitcast(F32), ALU.subtract)
                hent_ps = qps.tile([NE, SLAB], F32)
                for m in range(NM):
                    csl = slice((m % 4) * SLAB, (m % 4 + 1) * SLAB)
                    nc.tensor.matmul(
                        hent_ps[:], cmbt[0:NT, m * NE : (m + 1) * NE],
                        acc_sb[m // 4][0:NT, csl],
                        start=(m == 0), stop=False)
                    nc.tensor.matmul(
                        hent_ps[:], cmbt[0:NT, m * NE : (m + 1) * NE],
                        acc_lo[m // 4][0:NT, csl],
                        start=False, stop=(m == NM - 1))
                hent = qp.tile([NE, SLAB], F32)
                nc.scalar.copy(hent[:], hent_ps[:])

                # transpose to [anchor, entry]; both halves side by side
                hb = qp.tile([P, 2 * NE], F32)  # ih-major: [0:48]=ih0, [48:96]=ih1
                for ih in range(2):
                    ht_ps = qps.tile([P, NE], F32)
                    nc.tensor.transpose(ht_ps[:], hent[:, ih * P : (ih + 1) * P],
                                        ident[0:NE, 0:NE])
                    nc.scalar.copy(hb[:, ih * NE : (ih + 1) * NE], ht_ps[:])

                # ---------------- Gauss-Jordan (both halves packed) --------
                # view cols as (ih, e) with e = r*7+c over 42 used entries
                def hbv(sl):
                    return hb[:].rearrange("p (i e) -> p i e", i=2)[:, :, sl]
                piv = qp.tile([P, 2], F32)
                f12 = qp.tile([P, 12], F32)
                upd = qp.tile([P, 84], F32)
                f12v = f12[:].rearrange("p (i r) -> p i r", i=2)
                updv = upd[:].rearrange("p (i r c) -> p i r c", r=6, c=7)
                for j in range(6):
                    nc.vector.reciprocal(piv[:], hb[:, 8 * j : 2 * NE : NE])
                    nc.vector.tensor_tensor(
                        f12v, hbv(slice(j, 42, 7)),
                        piv[:].to_broadcast((P, 2, 6)), ALU.mult)
                    nc.vector.memset(f12[:, j : 12 : 6], 0.0)
                    nc.vector.tensor_tensor(
                        updv, f12v.to_broadcast((P, 2, 6, 7)),
                        hbv(slice(7 * j, 7 * j + 7)).unsqueeze(2).to_broadcast((P, 2, 6, 7)),
                        ALU.mult)
                    hview = hbv(slice(0, 42)).rearrange("p i (r c) -> p i r c", c=7)
                    nc.vector.tensor_tensor(hview, hview, updv, ALU.subtract)
                dinv = qp.tile([P, 12], F32)
                delta = qp.tile([P, 12], F32)
                dinvv = dinv[:].rearrange("p (i r) -> p i r", i=2)
                deltav = delta[:].rearrange("p (i r) -> p i r", i=2)
                nc.vector.reciprocal(dinvv, hbv(slice(0, 42, 8)))
                nc.vector.tensor_tensor(deltav, hbv(slice(6, 42, 7)), dinvv, ALU.mult)

                # ---------------- exp map coefficients (packed) ------------
                wsq = qp.tile([P, 6], F32)
                th2 = qp.tile([P, 2], F32)
                th = qp.tile([P, 2], F32)
                wv = deltav[:, :, 3:6]
                wsqv = wsq[:].rearrange("p (i r) -> p i r", i=2)
                nc.vector.tensor_tensor(wsqv, wv, wv, ALU.mult)
                nc.vector.tensor_reduce(th2[:], wsqv, AX.X, ALU.add)
                nc.scalar.sqrt(th[:], th2[:])
                mask = qp.tile([P, 2], F32)
                maski = qp.tile([P, 2], mybir.dt.int32)
                safe = qp.tile([P, 2], F32)
                nc.vector.tensor_scalar(mask[:], th[:], 1e-4, None, ALU.is_lt)
                nc.vector.tensor_copy(maski[:], mask[:])
                nc.vector.tensor_tensor(safe[:], th[:], mask[:], ALU.add)
                invs = qp.tile([P, 2], F32)
                invs2 = qp.tile([P, 2], F32)
                invs3 = qp.tile([P, 2], F32)
                nc.vector.reciprocal(invs[:], safe[:])
                nc.vector.tensor_tensor(invs2[:], invs[:], invs[:], ALU.mult)
                nc.vector.tensor_tensor(invs3[:], invs2[:], invs[:], ALU.mult)
                sh = qp.tile([P, 2], F32)
                sh2 = qp.tile([P, 2], F32)
                csf = qp.tile([P, 2], F32)
                snf = qp.tile([P, 2], F32)
                nc.scalar.activation(sh[:], safe[:], AF.Sin, scale=0.5)
                nc.vector.tensor_tensor(sh2[:], sh[:], sh[:], ALU.mult)
                nc.vector.tensor_scalar(csf[:], sh2[:], -2.0, 1.0, ALU.mult, ALU.add)
                nc.vector.tensor_scalar(snf[:], sh2[:], -1.0, 1.0, ALU.mult, ALU.add)
                nc.scalar.sqrt(snf[:], snf[:])
                nc.vector.tensor_tensor(snf[:], snf[:], sh[:], ALU.mult)
                nc.vector.tensor_scalar(snf[:], snf[:], 2.0, None, ALU.mult)
                abc = qp.tile([P, 6], F32)   # col = coeff(A,B,C)*2 + ih
                abct = qp.tile([P, 6], F32)
                tmp2 = qp.tile([P, 2], F32)
                nc.vector.tensor_tensor(abc[:, 0:2], snf[:], invs[:], ALU.mult)
                nc.vector.tensor_scalar(tmp2[:], csf[:], -1.0, 1.0, ALU.mult, ALU.add)
                nc.vector.tensor_tensor(abc[:, 2:4], tmp2[:], invs2[:], ALU.mult)
                nc.vector.tensor_tensor(tmp2[:], safe[:], snf[:], ALU.subtract)
                nc.vector.tensor_tensor(abc[:, 4:6], tmp2[:], invs3[:], ALU.mult)
                nc.vector.tensor_scalar(abct[:, 0:2], th2[:], -1.0 / 6.0, 1.0, ALU.mult, ALU.add)
                nc.vector.tensor_scalar(abct[:, 2:4], th2[:], -1.0 / 24.0, 0.5, ALU.mult, ALU.add)
                nc.vector.tensor_scalar(abct[:, 4:6], th2[:], -1.0 / 120.0, 1.0 / 6.0, ALU.mult, ALU.add)
                mask6 = qp.tile([P, 6], mybir.dt.int32)
                nc.vector.tensor_copy(
                    mask6[:].rearrange("p (a i) -> p a i", i=2),
                    maski[:].unsqueeze(1).to_broadcast((P, 3, 2)))
                nc.vector.copy_predicated(abc[:], mask6[:], abct[:])

                # ------- packed both-half R/V, translation, compose --------
                # layouts: delta [P,12] (ih-major r), wsq [P,6] (ih,r),
                # abc [P,6] (coeff*2+ih), th2 [P,2]
                def iv(tile_ap, n):
                    return tile_ap.rearrange("p (i e) -> p i e", i=n)
                wb = deltav[:, :, 3:6]          # (P,2,3)
                vb = deltav[:, :, 0:3]
                u3 = qp.tile([P, 6], F32)       # (ih, r)
                u3v = iv(u3[:], 2)
                nc.vector.tensor_tensor(
                    u3v, wsqv, th2[:].unsqueeze(2).to_broadcast((P, 2, 3)),
                    ALU.subtract)
                Aw = qp.tile([P, 6], F32)
                Bw = qp.tile([P, 6], F32)
                Cw = qp.tile([P, 6], F32)
                dB = qp.tile([P, 6], F32)
                dC = qp.tile([P, 6], F32)
                nc.vector.tensor_tensor(
                    iv(Aw[:], 2), wb,
                    abc[:, 0:2].unsqueeze(2).to_broadcast((P, 2, 3)), ALU.mult)
                nc.vector.tensor_tensor(
                    iv(Bw[:], 2), wb,
                    abc[:, 2:4].unsqueeze(2).to_broadcast((P, 2, 3)), ALU.mult)
                nc.vector.tensor_tensor(
                    iv(Cw[:], 2), wb,
                    abc[:, 4:6].unsqueeze(2).to_broadcast((P, 2, 3)), ALU.mult)
                nc.vector.tensor_tensor(
                    iv(dB[:], 2), u3v,
                    abc[:, 2:4].unsqueeze(2).to_broadcast((P, 2, 3)), ALU.mult)
                nc.vector.tensor_tensor(
                    iv(dC[:], 2), u3v,
                    abc[:, 4:6].unsqueeze(2).to_broadcast((P, 2, 3)), ALU.mult)
                # w components for both halves: delta cols 3+r and 9+r
                def wcol(r):
                    return delta[:, 3 + r : 12 : 6]
                qb = qp.tile([P, 6], F32)   # q01,q02,q12 x (2 ih): col=q*2+ih
                cb = qp.tile([P, 6], F32)
                nc.vector.tensor_tensor(qb[:, 0:2], Bw[:, 0:6:3], wcol(1), ALU.mult)
                nc.vector.tensor_tensor(qb[:, 2:4], Bw[:, 0:6:3], wcol(2), ALU.mult)
                nc.vector.tensor_tensor(qb[:, 4:6], Bw[:, 1:6:3], wcol(2), ALU.mult)
                nc.vector.tensor_tensor(cb[:, 0:2], Cw[:, 0:6:3], wcol(1), ALU.mult)
                nc.vector.tensor_tensor(cb[:, 2:4], Cw[:, 0:6:3], wcol(2), ALU.mult)
                nc.vector.tensor_tensor(cb[:, 4:6], Cw[:, 1:6:3], wcol(2), ALU.mult)

                # Rt/Vt: [P,18], col = entry*2 + ih
                Rt = qp.tile([P, 18], F32)
                Vt = qp.tile([P, 18], F32)
                for M, hat, dgc, oc in ((Rt, Aw, dB, qb), (Vt, Bw, dC, cb)):
                    nc.vector.tensor_scalar(M[:, 0:2], dgc[:, 0:6:3], 1.0, None, ALU.add)
                    nc.vector.tensor_tensor(M[:, 2:4], oc[:, 0:2], hat[:, 2:6:3], ALU.subtract)
                    nc.vector.tensor_tensor(M[:, 4:6], oc[:, 2:4], hat[:, 1:6:3], ALU.add)
                    nc.vector.tensor_tensor(M[:, 6:8], oc[:, 0:2], hat[:, 2:6:3], ALU.add)
                    nc.vector.tensor_scalar(M[:, 8:10], dgc[:, 1:6:3], 1.0, None, ALU.add)
                    nc.vector.tensor_tensor(M[:, 10:12], oc[:, 4:6], hat[:, 0:6:3], ALU.subtract)
                    nc.vector.tensor_tensor(M[:, 12:14], oc[:, 2:4], hat[:, 1:6:3], ALU.subtract)
                    nc.vector.tensor_tensor(M[:, 14:16], oc[:, 4:6], hat[:, 0:6:3], ALU.add)
                    nc.vector.tensor_scalar(M[:, 16:18], dgc[:, 2:6:3], 1.0, None, ALU.add)

                # translation t = V @ v  -> tvb [P,6] col = r*2 + ih
                tvb = qp.tile([P, 6], F32)
                trow = qp.tile([P, 6], F32)
                for r in range(3):
                    vrow = Vt[:, 6 * r : 6 * r + 6].rearrange("p (c i) -> p i c", i=2)
                    nc.vector.tensor_tensor(iv(trow[:], 2), vrow, vb, ALU.mult)
                    nc.vector.tensor_reduce(tvb[:, 2 * r : 2 * r + 2], iv(trow[:], 2),
                                            AX.X, ALU.add)

                # compose out = dT @ Tmat, packed [P, 32] (ih-major)
                tmib = qp.tile([P, 32], F32)
                nc.vector.tensor_copy(tmib[:, 0:16], tmi0[:])
                nc.vector.tensor_copy(tmib[:, 16:32], tmi1[:])
                tmv = iv(tmib[:], 2)            # (P,2,16)
                Ob = qp.tile([P, 32], F32)
                Obv = iv(Ob[:], 2)
                oac = qp.tile([P, 8], F32)
                oacv = iv(oac[:], 2)
                for r in range(3):
                    orow = Obv[:, :, 4 * r : 4 * r + 4]
                    nc.vector.tensor_tensor(
                        orow, tmv[:, :, 0:4],
                        Rt[:, 6 * r : 6 * r + 2].unsqueeze(2).to_broadcast((P, 2, 4)),
                        ALU.mult)
                    nc.vector.tensor_tensor(
                        oacv, tmv[:, :, 4:8],
                        Rt[:, 6 * r + 2 : 6 * r + 4].unsqueeze(2).to_broadcast((P, 2, 4)),
                        ALU.mult)
                    nc.vector.tensor_tensor(orow, orow, oacv, ALU.add)
                    nc.vector.tensor_tensor(
                        oacv, tmv[:, :, 8:12],
                        Rt[:, 6 * r + 4 : 6 * r + 6].unsqueeze(2).to_broadcast((P, 2, 4)),
                        ALU.mult)
                    nc.vector.tensor_tensor(orow, orow, oacv, ALU.add)
                    nc.vector.tensor_tensor(
                        oacv, tmv[:, :, 12:16],
                        tvb[:, 2 * r : 2 * r + 2].unsqueeze(2).to_broadcast((P, 2, 4)),
                        ALU.mult)
                    nc.vector.tensor_tensor(orow, orow, oacv, ALU.add)
                nc.vector.tensor_copy(Obv[:, :, 12:16], tmv[:, :, 12:16])
                nc.sync.dma_start(out_d[0:P, :], Ob[:, 0:16])
                nc.sync.dma_start(out_d[P : 2 * P, :], Ob[:, 16:32])

    nc.compile()
    return nc


def prep_inputs(embeddings, revisions, weights, depth, pix_T_camXs, Tmat):
    f = np.float32
    emb = np.ascontiguousarray(embeddings, dtype=f).reshape(B, C, N)
    rev = np.ascontiguousarray(revisions, dtype=f).reshape(B, 3, N)
    wgt = np.ascontiguousarray(weights, dtype=f).reshape(B, 3, N)
    dep = np.ascontiguousarray(depth, dtype=f).reshape(B, N)
    pix = np.ascontiguousarray(pix_T_camXs, dtype=f)
    tm = np.ascontiguousarray(Tmat, dtype=f).reshape(B, N, 16)

    ys, xs = np.meshgrid(np.arange(H, dtype=f), np.arange(W, dtype=f), indexing="ij")
    u = xs.reshape(-1)
    v = ys.reshape(-1)

    def kmaj(a):
        a = a.reshape(KC, P, -1).transpose(1, 0, 2)
        return np.ascontiguousarray(a.reshape(P, -1), dtype=f)

    uk = kmaj(u)
    vk = kmaj(v)
    cmb = combo_matrices()
    in_maps = []
    for core in range(NCORES):
        b = core // 4
        s0 = (core % 4) * SLAB
        fx, fy, x0, y0 = pix[b, 0, 0], pix[b, 1, 1], pix[b, 0, 2], pix[b, 1, 2]
        tms = tm[b][s0 : s0 + SLAB]       # [256, 16]
        # joint moving operand [21, 1024]:
        # cols 0:256 Xp, 256:512 Yp, 512:768 Zp, 768:1024 s (affinity arg)
        # rows 0:16 pair with -2e_k; 16:19 with x,y,z; 19 with ones; 20 with ek2
        rhsj = np.zeros((21, 4 * SLAB), f)
        for p in range(3):
            for q in range(3):
                rhsj[C + q, p * SLAB : (p + 1) * SLAB] = tms[:, 4 * p + q]
            rhsj[C + 3, p * SLAB : (p + 1) * SLAB] = tms[:, 4 * p + 3]
        rhsj[0:C, 3 * SLAB : 4 * SLAB] = emb[b][:, s0 : s0 + SLAB]
        rhsj[C + 4, 3 * SLAB : 4 * SLAB] = 1.0
        # row C+3 (ones multiplier) cols 768:1024 = ei2, filled on device
        kblob = np.concatenate(
            [kmaj(dep[b]), uk, vk, kmaj(tm[b]), kmaj(rev[b].T), kmaj(wgt[b].T)], 1)
        in_maps.append({
            "kblob": np.ascontiguousarray(kblob),
            "ek": emb[b],
            "ei": np.ascontiguousarray(emb[b][:, s0 : s0 + SLAB]),
            "uvzz": np.ascontiguousarray(np.stack([u, v], 0)),
            "zz2": np.ascontiguousarray(np.stack([dep[b], dep[b]], 0)),
            "zo": np.ascontiguousarray(np.stack([dep[b], np.ones(N, f)], 0)),
            "cam": np.array([[fx, x0, fy, y0]], dtype=f),
            "cam2": np.array([[x0, fx], [y0, fy]], dtype=f),
            "rhsj": rhsj,
            "cmbt": cmb,
            "tm_i": np.ascontiguousarray(tms),
        })
    return in_maps


def gather_output(results):
    full = np.empty((B, N, 16), dtype=np.float32)
    for core in range(NCORES):
        b = core // 4
        s0 = (core % 4) * SLAB
        full[b, s0 : s0 + SLAB] = results[core]["out"]
    return full.reshape(B, H, W, 4, 4)


_NC_CACHE = {}


def kernel(**inputs):
    if "nc" not in _NC_CACHE:
        _NC_CACHE["nc"] = build_nc()
    nc = _NC_CACHE["nc"]
    in_maps = prep_inputs(**inputs)
    res = run_bass_kernel_spmd(nc, in_maps, core_ids=list(range(NCORES)))
    return gather_output(res.results)



# revision 12
# speedup vs baseline: 1.3911x; 1.3911x over previous
"""Dense SE(3) Gauss-Newton kernel for Trainium2, sharded over 8 NeuronCores.

Sharding: core owns batch b = core//4 and a 256-anchor slab of the i axis;
the k axis (1024) runs in 8 chunks of 128 on the partition dimension with
anchors on the free dimension.

Per (i,k) the kernel materializes 13 fp16 "moving bands" (powers of the
projected-point deltas d, dX, dY weighted by the embedding affinity); the
6x6 normal equations + rhs are then accumulated straight into PSUM as 27
entry rows by matmuls against host-precomputed per-k fp16 coefficient
tables (per-entry stationaries).  The residual is decomposed around each
point's self-projection (delta form) so every band is cancellation-free
and fp16-safe.  All transcendentals (1/Z, sqrt, exp) are computed on the
Scalar engine from the single natural_log_exp activation-table set, so
the kernel performs exactly one ACT_TABLE_LOAD.
"""
import sys

sys.path.insert(0, "/opt/trn_rl_repo")

import numpy as np

from concourse import bacc, tile
import concourse.mybir as mybir
import concourse.bass_utils as _bu
from concourse.bass_utils import run_bass_kernel_spmd

F32 = mybir.dt.float32
F16 = mybir.dt.float16
AF = mybir.ActivationFunctionType
ALU = mybir.AluOpType
AX = mybir.AxisListType

B, C, H, W = 2, 16, 32, 32
N = H * W
NCORES = 8
SLAB = 256
KC = 8
P = 128
GR = 29                       # geometry contraction rows
HTRI = [(p, q) for p in range(6) for q in range(p, 6)]  # 21 entries
NHB = 6                       # Hm bands: A, AdX, AdY, AdX2, AdY2, Ad2
NRB = 7                       # rhs bands: d, d2, ddX, ddY, ddX2, ddY2, d2dD
ACC_CW = NHB * 21 + NRB * 6   # 168 stationary cols per chunk
NE = 48                       # augmented 6x7 system padded


def build_nc():
    nc = bacc.Bacc("TRN2", target_bir_lowering=False, debug=False)

    geom_d = nc.dram_tensor("geom", [GR, 4096], F16, kind="ExternalInput")
    accst_d = nc.dram_tensor("accst", [P, KC * ACC_CW], F16, kind="ExternalInput")
    misc_d = nc.dram_tensor("misc", [P, 136], F32, kind="ExternalInput")
    out_d = nc.dram_tensor("out", [P, 32], F32, kind="ExternalOutput")

    with tile.TileContext(nc) as tc:
        with tc.tile_pool(name="persist", bufs=1) as pp, \
             tc.tile_pool(name="acc_ps", bufs=1, space="PSUM") as accp:

            geom = pp.tile([GR, 4096], F16)
            accst = pp.tile([P, KC * ACC_CW], F16)
            misc = pp.tile([P, 136], F32)
            nc.sync.dma_start(geom[:], geom_d[:])
            nc.sync.dma_start(accst[:], accst_d[:])
            nc.sync.dma_start(misc[:], misc_d[:])
            stat_hi = geom[:, 0:1024]
            stat_lo = geom[:, 1024:2048]
            mov_hi = geom[:, 2048:3072]
            mov_lo = geom[:, 3072:4096]
            dkk = misc[:, 0:8]
            tmi0 = misc[:, 8:24]
            tmi1 = misc[:, 24:40]
            EXPh = misc[:, 40:88]          # rows 0:21: Hm-entry -> 6x7 expander
            EXPr = misc[:, 88:136]         # rows 0:6: rhs-entry -> col 6

            lnbias = pp.tile([P, 1], F32)
            nc.vector.memset(lnbias[:], 1e-12)
            sallh = pp.tile([P, 2048], F16)   # ||e_i-e_k||^2, relu'd
            affh = pp.tile([P, 2048], F16)    # exp(-||e_i-e_k||)
            atmp = pp.tile([P, 1024], F32)
            dall = pp.tile([P, 2048], F32)    # d = 1/Zp
            dhall = pp.tile([P, 2048], F16)
            d2all = pp.tile([P, 2048], F16)
            dXall = pp.tile([P, 2048], F16)   # dX = Xp/Zp - Xkk
            dYall = pp.tile([P, 2048], F16)

            accH = accp.tile([21, SLAB], F32)
            accR = accp.tile([6, SLAB], F32)

            with tc.tile_pool(name="mm_ps", bufs=2, space="PSUM") as mmp, \
                 tc.tile_pool(name="work", bufs=2) as wp:

                def pass_a(c):
                    ck = slice(c * P, (c + 1) * P)
                    cs = slice(c * SLAB, (c + 1) * SLAB)
                    Zs = mmp.tile([P, 2 * SLAB], F32, name=f"Zs{c}", tag="Zs")
                    XY = mmp.tile([P, 2 * SLAB], F32, name=f"XY{c}", tag="XY")
                    sh = stat_hi[:, ck]
                    sl = stat_lo[:, ck]
                    nc.tensor.matmul(Zs[:], sh, mov_hi[:, 512:1024], start=True, stop=False)
                    nc.tensor.matmul(Zs[:], sh, mov_lo[:, 512:1024], start=False, stop=False)
                    nc.tensor.matmul(XY[:], sh, mov_hi[:, 0:512], start=True, stop=False)
                    nc.tensor.matmul(XY[:], sh, mov_lo[:, 0:512], start=False, stop=False)
                    nc.tensor.matmul(Zs[:], sl, mov_hi[:, 512:1024], start=False, stop=True)
                    nc.tensor.matmul(XY[:], sl, mov_hi[:, 0:512], start=False, stop=True)
                    lnz = wp.tile([P, SLAB], F32, name=f"lnz{c}", tag="lnz")
                    d32 = dall[:, cs]
                    nc.scalar.activation(lnz[:], Zs[:, 0:SLAB], AF.Ln)
                    nc.scalar.activation(d32, lnz[:], AF.Exp, scale=-1.0)
                    nc.scalar.activation(d2all[:, cs], lnz[:], AF.Exp, scale=-2.0)
                    nc.scalar.activation(sallh[:, cs], Zs[:, SLAB : 2 * SLAB], AF.Relu)
                    nc.vector.tensor_tensor(dXall[:, cs], XY[:, 0:SLAB], d32, ALU.mult)
                    nc.vector.tensor_tensor(dYall[:, cs], XY[:, SLAB : 2 * SLAB], d32, ALU.mult)
                    nc.vector.tensor_copy(dhall[:, cs], d32)

                def aff_batch(h):
                    hs = slice(h * 4 * SLAB, (h + 1) * 4 * SLAB)
                    nc.scalar.activation(atmp[:], sallh[:, hs], AF.Ln, bias=lnbias[:])
                    nc.scalar.activation(atmp[:], atmp[:], AF.Exp, scale=0.5)
                    nc.scalar.activation(affh[:, hs], atmp[:], AF.Exp, scale=-1.0)

                def pass_b(c):
                    cs = slice(c * SLAB, (c + 1) * SLAB)
                    co = c * ACC_CW
                    affc = affh[:, cs]
                    dXt = dXall[:, cs]
                    dYt = dYall[:, cs]
                    dh = dhall[:, cs]
                    d2h = d2all[:, cs]
                    d32 = dall[:, cs]

                    def wt(nm):
                        return wp.tile([P, SLAB], F16, name=f"{nm}{c}", tag=nm)

                    bA = wt("bA")
                    bAdX = wt("bAdX")
                    bAdY = wt("bAdY")
                    bdX = wt("bdX")
                    bdY = wt("bdY")
                    bd2dD = wt("bd2dD")
                    bdX2 = wt("bdX2")
                    bdY2 = wt("bdY2")
                    bAdX2 = wt("bAdX2")
                    bAdY2 = wt("bAdY2")
                    bAd2 = wt("bAd2")
                    nc.vector.tensor_tensor(bA[:], affc, d2h, ALU.mult)
                    nc.vector.tensor_tensor(bAdX[:], bA[:], dXt, ALU.mult)
                    nc.vector.tensor_tensor(bAdY[:], bA[:], dYt, ALU.mult)
                    nc.vector.tensor_tensor(bdX[:], dh, dXt, ALU.mult)
                    nc.vector.tensor_tensor(bdY[:], dh, dYt, ALU.mult)
                    nc.vector.scalar_tensor_tensor(bd2dD[:], d32, dkk[:, c : c + 1],
                                                   d2h, ALU.subtract, ALU.mult)
                    nc.vector.tensor_tensor(bdX2[:], bdX[:], dXt, ALU.mult)
                    nc.vector.tensor_tensor(bdY2[:], bdY[:], dYt, ALU.mult)
                    nc.gpsimd.tensor_tensor(bAdX2[:], bAdX[:], dXt, ALU.mult)
                    nc.gpsimd.tensor_tensor(bAdY2[:], bAdY[:], dYt, ALU.mult)
                    nc.gpsimd.tensor_tensor(bAd2[:], bA[:], d2h, ALU.mult)

                    hbands = [bA[:], bAdX[:], bAdY[:], bAdX2[:], bAdY2[:], bAd2[:]]
                    for m, bt in enumerate(hbands):
                        nc.tensor.matmul(
                            accH[:], accst[:, co + m * 21 : co + (m + 1) * 21], bt,
                            start=(c == 0 and m == 0),
                            stop=(c == KC - 1 and m == NHB - 1))
                    rbands = [dh, d2h, bdX[:], bdY[:], bdX2[:], bdY2[:], bd2dD[:]]
                    ro = co + NHB * 21
                    for m, bt in enumerate(rbands):
                        nc.tensor.matmul(
                            accR[:], accst[:, ro + m * 6 : ro + (m + 1) * 6], bt,
                            start=(c == 0 and m == 0),
                            stop=(c == KC - 1 and m == NRB - 1))

                for c in range(4):
                    pass_a(c)
                aff_batch(0)
                for c in range(4, KC):
                    pass_a(c)
                for c in range(4):
                    pass_b(c)
                aff_batch(1)
                for c in range(4, KC):
                    pass_b(c)

            # ---------------- solve / exp map / compose -----------------
            with tc.tile_pool(name="post", bufs=2) as qp, \
                 tc.tile_pool(name="post_ps", bufs=2, space="PSUM") as qps:
                acc_sbH = qp.tile([21, SLAB], F32)
                acc_sbR = qp.tile([6, SLAB], F32)
                nc.scalar.copy(acc_sbH[:], accH[:])
                nc.scalar.copy(acc_sbR[:], accR[:])

                # expand 27 entry rows -> [anchor, 6x7 augmented] per half
                hb = qp.tile([P, 2 * NE], F32)  # ih-major: [0:48]=ih0, [48:96]=ih1
                for ih in range(2):
                    hb_ps = qps.tile([P, NE], F32, name=f"hbps{ih}", tag="hbps")
                    nc.tensor.matmul(hb_ps[:], acc_sbH[:, ih * P : (ih + 1) * P],
                                     EXPh[0:21, :], start=True, stop=False)
                    nc.tensor.matmul(hb_ps[:], acc_sbR[:, ih * P : (ih + 1) * P],
                                     EXPr[0:6, :], start=False, stop=True)
                    nc.scalar.copy(hb[:, ih * NE : (ih + 1) * NE], hb_ps[:])

                # ---------------- Gauss-Jordan (both halves packed) --------
                def hbv(sl):
                    return hb[:].rearrange("p (i e) -> p i e", i=2)[:, :, sl]
                piv = qp.tile([P, 2], F32)
                f12 = qp.tile([P, 12], F32)
                upd = qp.tile([P, 84], F32)
                f12v = f12[:].rearrange("p (i r) -> p i r", i=2)
                updv = upd[:].rearrange("p (i r c) -> p i r c", r=6, c=7)
                for j in range(6):
                    nc.vector.reciprocal(piv[:], hb[:, 8 * j : 2 * NE : NE])
                    nc.vector.tensor_tensor(
                        f12v, hbv(slice(j, 42, 7)),
                        piv[:].to_broadcast((P, 2, 6)), ALU.mult)
                    nc.vector.memset(f12[:, j : 12 : 6], 0.0)
                    nc.vector.tensor_tensor(
                        updv, f12v.to_broadcast((P, 2, 6, 7)),
                        hbv(slice(7 * j, 7 * j + 7)).unsqueeze(2).to_broadcast((P, 2, 6, 7)),
                        ALU.mult)
                    hview = hbv(slice(0, 42)).rearrange("p i (r c) -> p i r c", c=7)
                    nc.vector.tensor_tensor(hview, hview, updv, ALU.subtract)
                dinv = qp.tile([P, 12], F32)
                delta = qp.tile([P, 12], F32)
                dinvv = dinv[:].rearrange("p (i r) -> p i r", i=2)
                deltav = delta[:].rearrange("p (i r) -> p i r", i=2)
                nc.vector.reciprocal(dinvv, hbv(slice(0, 42, 8)))
                nc.vector.tensor_tensor(deltav, hbv(slice(6, 42, 7)), dinvv, ALU.mult)

                # ------------- exp map coefficients via Taylor in th^2 -----
                wsq = qp.tile([P, 6], F32)
                th2 = qp.tile([P, 2], F32)
                wv = deltav[:, :, 3:6]
                vb = deltav[:, :, 0:3]
                wsqv = wsq[:].rearrange("p (i r) -> p i r", i=2)
                nc.vector.tensor_tensor(wsqv, wv, wv, ALU.mult)
                nc.vector.tensor_reduce(th2[:], wsqv, AX.X, ALU.add)
                tu2 = qp.tile([P, 2], F32)
                tu3 = qp.tile([P, 2], F32)
                nc.vector.tensor_tensor(tu2[:], th2[:], th2[:], ALU.mult)
                nc.vector.tensor_tensor(tu3[:], tu2[:], th2[:], ALU.mult)
                abc = qp.tile([P, 6], F32)   # col = coeff(A,B,C)*2 + ih
                t6 = qp.tile([P, 6], F32)
                # A = sin(t)/t, B = (1-cos t)/t^2, C = (t - sin t)/t^3 series
                nc.vector.tensor_scalar(t6[:, 0:2], th2[:], -1.0 / 6.0, 1.0, ALU.mult, ALU.add)
                nc.vector.tensor_scalar(t6[:, 2:4], th2[:], -1.0 / 24.0, 0.5, ALU.mult, ALU.add)
                nc.vector.tensor_scalar(t6[:, 4:6], th2[:], -1.0 / 120.0, 1.0 / 6.0, ALU.mult, ALU.add)
                nc.vector.scalar_tensor_tensor(abc[:, 0:2], tu2[:], 1.0 / 120.0,
                                               t6[:, 0:2], ALU.mult, ALU.add)
                nc.vector.scalar_tensor_tensor(abc[:, 2:4], tu2[:], 1.0 / 720.0,
                                               t6[:, 2:4], ALU.mult, ALU.add)
                nc.vector.scalar_tensor_tensor(abc[:, 4:6], tu2[:], 1.0 / 5040.0,
                                               t6[:, 4:6], ALU.mult, ALU.add)
                nc.vector.scalar_tensor_tensor(abc[:, 0:2], tu3[:], -1.0 / 5040.0,
                                               abc[:, 0:2], ALU.mult, ALU.add)
                nc.vector.scalar_tensor_tensor(abc[:, 2:4], tu3[:], -1.0 / 40320.0,
                                               abc[:, 2:4], ALU.mult, ALU.add)
                nc.vector.scalar_tensor_tensor(abc[:, 4:6], tu3[:], -1.0 / 362880.0,
                                               abc[:, 4:6], ALU.mult, ALU.add)

                # ------- packed both-half R/V, translation, compose --------
                def iv(tile_ap, n):
                    return tile_ap.rearrange("p (i e) -> p i e", i=n)
                u3 = qp.tile([P, 6], F32)       # (ih, r): w_r^2 - th^2
                u3v = iv(u3[:], 2)
                nc.vector.tensor_tensor(
                    u3v, wsqv, th2[:].unsqueeze(2).to_broadcast((P, 2, 3)),
                    ALU.subtract)
                Aw = qp.tile([P, 6], F32)
                Bw = qp.tile([P, 6], F32)
                Cw = qp.tile([P, 6], F32)
                dB = qp.tile([P, 6], F32)
                dC = qp.tile([P, 6], F32)
                nc.vector.tensor_tensor(
                    iv(Aw[:], 2), wv,
                    abc[:, 0:2].unsqueeze(2).to_broadcast((P, 2, 3)), ALU.mult)
                nc.vector.tensor_tensor(
                    iv(Bw[:], 2), wv,
                    abc[:, 2:4].unsqueeze(2).to_broadcast((P, 2, 3)), ALU.mult)
                nc.vector.tensor_tensor(
                    iv(Cw[:], 2), wv,
                    abc[:, 4:6].unsqueeze(2).to_broadcast((P, 2, 3)), ALU.mult)
                nc.vector.tensor_tensor(
                    iv(dB[:], 2), u3v,
                    abc[:, 2:4].unsqueeze(2).to_broadcast((P, 2, 3)), ALU.mult)
                nc.vector.tensor_tensor(
                    iv(dC[:], 2), u3v,
                    abc[:, 4:6].unsqueeze(2).to_broadcast((P, 2, 3)), ALU.mult)

                def wcol(r):
                    return delta[:, 3 + r : 12 : 6]
                qb = qp.tile([P, 6], F32)   # q01,q02,q12 x (2 ih): col=q*2+ih
                cb = qp.tile([P, 6], F32)
                nc.vector.tensor_tensor(qb[:, 0:2], Bw[:, 0:6:3], wcol(1), ALU.mult)
                nc.vector.tensor_tensor(qb[:, 2:4], Bw[:, 0:6:3], wcol(2), ALU.mult)
                nc.vector.tensor_tensor(qb[:, 4:6], Bw[:, 1:6:3], wcol(2), ALU.mult)
                nc.vector.tensor_tensor(cb[:, 0:2], Cw[:, 0:6:3], wcol(1), ALU.mult)
                nc.vector.tensor_tensor(cb[:, 2:4], Cw[:, 0:6:3], wcol(2), ALU.mult)
                nc.vector.tensor_tensor(cb[:, 4:6], Cw[:, 1:6:3], wcol(2), ALU.mult)

                # Rt/Vt: [P,18], col = entry*2 + ih
                Rt = qp.tile([P, 18], F32)
                Vt = qp.tile([P, 18], F32)
                for M, hat, dgc, oc in ((Rt, Aw, dB, qb), (Vt, Bw, dC, cb)):
                    nc.vector.tensor_scalar(M[:, 0:2], dgc[:, 0:6:3], 1.0, None, ALU.add)
                    nc.vector.tensor_tensor(M[:, 2:4], oc[:, 0:2], hat[:, 2:6:3], ALU.subtract)
                    nc.vector.tensor_tensor(M[:, 4:6], oc[:, 2:4], hat[:, 1:6:3], ALU.add)
                    nc.vector.tensor_tensor(M[:, 6:8], oc[:, 0:2], hat[:, 2:6:3], ALU.add)
                    nc.vector.tensor_scalar(M[:, 8:10], dgc[:, 1:6:3], 1.0, None, ALU.add)
                    nc.vector.tensor_tensor(M[:, 10:12], oc[:, 4:6], hat[:, 0:6:3], ALU.subtract)
                    nc.vector.tensor_tensor(M[:, 12:14], oc[:, 2:4], hat[:, 1:6:3], ALU.subtract)
                    nc.vector.tensor_tensor(M[:, 14:16], oc[:, 4:6], hat[:, 0:6:3], ALU.add)
                    nc.vector.tensor_scalar(M[:, 16:18], dgc[:, 2:6:3], 1.0, None, ALU.add)

                # translation t = V @ v  -> tvb [P,6] col = r*2 + ih
                tvb = qp.tile([P, 6], F32)
                trow = qp.tile([P, 6], F32)
                for r in range(3):
                    vrow = Vt[:, 6 * r : 6 * r + 6].rearrange("p (c i) -> p i c", i=2)
                    nc.vector.tensor_tensor(iv(trow[:], 2), vrow, vb, ALU.mult)
                    nc.vector.tensor_reduce(tvb[:, 2 * r : 2 * r + 2], iv(trow[:], 2),
                                            AX.X, ALU.add)

                # compose out = dT @ Tmat, packed [P, 32] (ih-major)
                tmib = qp.tile([P, 32], F32)
                nc.vector.tensor_copy(tmib[:, 0:16], tmi0)
                nc.vector.tensor_copy(tmib[:, 16:32], tmi1)
                tmv = iv(tmib[:], 2)            # (P,2,16)
                Ob = qp.tile([P, 32], F32)
                Obv = iv(Ob[:], 2)
                oac = qp.tile([P, 8], F32)
                oacv = iv(oac[:], 2)
                for r in range(3):
                    orow = Obv[:, :, 4 * r : 4 * r + 4]
                    nc.vector.tensor_tensor(
                        orow, tmv[:, :, 0:4],
                        Rt[:, 6 * r : 6 * r + 2].unsqueeze(2).to_broadcast((P, 2, 4)),
                        ALU.mult)
                    nc.vector.tensor_tensor(
                        oacv, tmv[:, :, 4:8],
                        Rt[:, 6 * r + 2 : 6 * r + 4].unsqueeze(2).to_broadcast((P, 2, 4)),
                        ALU.mult)
                    nc.vector.tensor_tensor(orow, orow, oacv, ALU.add)
                    nc.vector.tensor_tensor(
                        oacv, tmv[:, :, 8:12],
                        Rt[:, 6 * r + 4 : 6 * r + 6].unsqueeze(2).to_broadcast((P, 2, 4)),
                        ALU.mult)
                    nc.vector.tensor_tensor(orow, orow, oacv, ALU.add)
                    nc.vector.tensor_tensor(
                        oacv, tmv[:, :, 12:16],
                        tvb[:, 2 * r : 2 * r + 2].unsqueeze(2).to_broadcast((P, 2, 4)),
                        ALU.mult)
                    nc.vector.tensor_tensor(orow, orow, oacv, ALU.add)
                nc.vector.tensor_copy(Obv[:, :, 12:16], tmv[:, :, 12:16])
                nc.sync.dma_start(out_d[:], Ob[:])

    nc.compile()
    return nc


def _split16(x):
    hi = np.asarray(x, np.float16)
    lo = np.asarray(x - hi.astype(np.float64), np.float16)
    return hi, lo


def prep_inputs(embeddings, revisions, weights, depth, pix_T_camXs, Tmat):
    f6 = np.float64
    emb = np.asarray(embeddings, f6).reshape(B, C, N)
    rev = np.asarray(revisions, f6).reshape(B, 3, N)
    wgt = np.asarray(weights, f6).reshape(B, 3, N)
    dep = np.asarray(depth, f6).reshape(B, N)
    pix = np.asarray(pix_T_camXs, f6)
    tm = np.asarray(Tmat, f6).reshape(B, N, 16)

    ys, xs = np.meshgrid(np.arange(H, dtype=f6), np.arange(W, dtype=f6),
                         indexing="ij")
    u = xs.reshape(-1)
    v = ys.reshape(-1)

    in_maps = []
    per_batch = []
    for b in range(B):
        fx, fy, x0, y0 = pix[b, 0, 0], pix[b, 1, 1], pix[b, 0, 2], pix[b, 1, 2]
        z = dep[b]
        X = (u - x0) * z / fx
        Y = (v - y0) * z / fy
        T = tm[b].reshape(N, 4, 4)
        R, t = T[:, :3, :3], T[:, :3, 3]
        xyz = np.stack([X, Y, z], -1)
        TjXj = np.einsum("kpq,kq->kp", R, xyz) + t
        Xkk = TjXj[:, 0] / TjXj[:, 2]
        Ykk = TjXj[:, 1] / TjXj[:, 2]
        dkk = 1.0 / TjXj[:, 2]
        on, zn = np.ones(N), np.zeros(N)
        JT0 = np.stack([on, zn, zn, zn, -z, Y], -1)
        JT1 = np.stack([zn, on, zn, z, zn, -X], -1)
        JT2 = np.stack([zn, zn, on, -Y, X, zn], -1)
        G0 = JT0 - Xkk[:, None] * JT2
        G1 = JT1 - Ykk[:, None] * JT2
        w0, w1, w2 = wgt[b, 0], wgt[b, 1], wgt[b, 2]
        r0, r1, r2 = rev[b, 0], rev[b, 1], rev[b, 2]

        def outer(a, bb):
            return np.einsum("kp,kq->kpq", a, bb)

        P00 = outer(G0, G0)
        P11 = outer(G1, G1)
        P22 = outer(JT2, JT2)
        P02 = outer(G0, JT2) + outer(JT2, G0)
        P12 = outer(G1, JT2) + outer(JT2, G1)
        wfx = (w0 * fx * fx)[:, None, None]
        wfy = (w1 * fy * fy)[:, None, None]
        SH = [wfx * P00 + wfy * P11, -wfx * P02, -wfy * P12,
              wfx * P22, wfy * P22, w2[:, None, None] * P22]
        SR = [-(fx * r0)[:, None] * G0 - (fy * r1)[:, None] * G1,
              r2[:, None] * JT2,
              fx * fx * G0 + (fx * r0)[:, None] * JT2,
              fy * fy * G1 + (fy * r1)[:, None] * JT2,
              -fx * fx * JT2,
              -fy * fy * JT2,
              -JT2]
        lam = 1.0 / (fx * fx)
        # acc stationary [128, KC*ACC_CW]; partition p of chunk c is k=c*128+p
        accst = np.zeros((P, KC * ACC_CW), f6)
        for c in range(KC):
            ks = slice(c * P, (c + 1) * P)
            co = c * ACC_CW
            for m, S in enumerate(SH):
                for ei, (p_, q_) in enumerate(HTRI):
                    accst[:, co + m * 21 + ei] = S[ks, p_, q_] * lam
            ro = co + NHB * 21
            for m, V in enumerate(SR):
                accst[:, ro + m * 6 : ro + (m + 1) * 6] = V[ks] * lam
        # geometry stationary [29, N]
        stat = np.zeros((GR, N), f6)
        stat[0:16] = -2.0 * emb[b]
        stat[16], stat[17], stat[18], stat[19] = X, Y, z, 1.0
        stat[20:24] = Xkk[None] * stat[16:20]
        stat[24:28] = Ykk[None] * stat[16:20]
        stat[28] = (emb[b] ** 2).sum(0)
        per_batch.append(dict(stat=stat, accst=accst, dkk=dkk, fx=fx,
                              emb=emb[b], T=T))

    cmbH = np.zeros((P, 48), np.float32)
    for ei, (p_, q_) in enumerate(HTRI):
        cmbH[ei, p_ * 7 + q_] = 1.0
        if p_ != q_:
            cmbH[ei, q_ * 7 + p_] = 1.0
    cmbR = np.zeros((P, 48), np.float32)
    for p_ in range(6):
        cmbR[p_, p_ * 7 + 6] = 1.0

    for core in range(NCORES):
        b = core // 4
        s0 = (core % 4) * SLAB
        pb = per_batch[b]
        T = pb["T"]
        # moving operand [29, 4*SLAB]: X' | Y' | Z | s blocks
        mov = np.zeros((GR, 4 * SLAB), f6)
        Ts = T[s0 : s0 + SLAB]
        for blk, row in ((0, 0), (1, 1), (2, 2)):
            mov[16:19, blk * SLAB : (blk + 1) * SLAB] = Ts[:, row, 0:3].T
            mov[19, blk * SLAB : (blk + 1) * SLAB] = Ts[:, row, 3]
        mov[20:23, 0:SLAB] = -Ts[:, 2, 0:3].T
        mov[23, 0:SLAB] = -Ts[:, 2, 3]
        mov[24:27, SLAB : 2 * SLAB] = -Ts[:, 2, 0:3].T
        mov[27, SLAB : 2 * SLAB] = -Ts[:, 2, 3]
        ei_ = pb["emb"][:, s0 : s0 + SLAB]
        mov[0:16, 3 * SLAB : 4 * SLAB] = ei_
        mov[19, 3 * SLAB : 4 * SLAB] = (ei_ ** 2).sum(0)
        mov[28, 3 * SLAB : 4 * SLAB] = 1.0

        sh, sl = _split16(pb["stat"])
        mh, ml = _split16(mov)
        geom = np.concatenate([sh, sl, mh, ml], 1)

        misc = np.zeros((P, 136), np.float32)
        misc[:, 0:8] = pb["dkk"].reshape(KC, P).T
        tms = np.asarray(tm[b][s0 : s0 + SLAB], np.float32)
        misc[:, 8:24] = tms[0:P]
        misc[:, 24:40] = tms[P : 2 * P]
        misc[:, 40:88] = cmbH
        misc[:, 88:136] = cmbR

        in_maps.append({
            "geom": np.ascontiguousarray(geom, np.float16),
            "accst": np.ascontiguousarray(pb["accst"], np.float16),
            "misc": np.ascontiguousarray(misc),
        })
    return in_maps


def gather_output(results):
    full = np.empty((B, N, 16), dtype=np.float32)
    for core in range(NCORES):
        b = core // 4
        s0 = (core % 4) * SLAB
        out = results[core]["out"]
        full[b, s0 : s0 + P] = out[:, 0:16]
        full[b, s0 + P : s0 + SLAB] = out[:, 16:32]
    return full.reshape(B, H, W, 4, 4)


_NC_CACHE = {}


def kernel(**inputs):
    if "nc" not in _NC_CACHE:
        _NC_CACHE["nc"] = build_nc()
    nc = _NC_CACHE["nc"]
    in_maps = prep_inputs(**inputs)
    res = run_bass_kernel_spmd(nc, in_maps, core_ids=list(range(NCORES)))
    return gather_output(res.results)


# revision 19
# speedup vs baseline: 1.7061x; 1.2264x over previous
"""Dense SE(3) Gauss-Newton kernel for Trainium2, sharded over 8 NeuronCores.

Sharding: core owns batch b = core//4 and a 256-anchor slab of the i axis;
the k axis (1024) runs in 8 chunks of 128 on the partition dimension with
anchors on the free dimension.

Per (i,k) the kernel materializes 13 fp16 "moving bands" (powers of the
projected-point deltas d, dX, dY, optionally weighted by the embedding
affinity); the 6x6 normal equations + rhs are accumulated straight into
PSUM as 27 entry rows by matmuls against host-precomputed per-k fp16
coefficient tables (per-entry stationaries).  The residual is decomposed
around each point's self-projection (delta form) so every band is
cancellation-free and fp16-safe.  The geometry inputs are pre-quantized
to fp16 on the host and every host-side constant (self-projections,
coefficient tables) is derived from the quantized values, which makes a
single-pass fp16 geometry matmul exact-enough by construction.  1/Z runs
on the Vector engine (fast custom reciprocal); the affinity
exp(-sqrt(s)) is batched once so only two ACT table switches happen.
"""
import sys

sys.path.insert(0, "/opt/trn_rl_repo")

import numpy as np

from concourse import bacc, tile
import concourse.mybir as mybir
from concourse.bass_utils import run_bass_kernel_spmd

F32 = mybir.dt.float32
F16 = mybir.dt.float16
AF = mybir.ActivationFunctionType
ALU = mybir.AluOpType
AX = mybir.AxisListType

B, C, H, W = 2, 16, 32, 32
N = H * W
NCORES = 8
SLAB = 256
KC = 8
P = 128
GR = 32                       # geometry contraction rows
HTRI = [(p, q) for p in range(6) for q in range(p, 6)]  # 21 entries
NHB = 6                       # Hm bands: A, AdX, AdY, AdX2, AdY2, Ad2
NRB = 7                       # rhs bands: d, d2, ddX, ddY, ddX2, ddY2, d2dD
ACC_CW = NHB * 21 + NRB * 6   # 168 stationary cols per chunk
NE = 48                       # augmented 6x7 system padded


def build_nc():
    nc = bacc.Bacc("TRN2", target_bir_lowering=False, debug=False)

    geom_d = nc.dram_tensor("geom", [GR, 2048], F16, kind="ExternalInput")
    accst_d = nc.dram_tensor("accst", [P, KC * ACC_CW], F16, kind="ExternalInput")
    misc_d = nc.dram_tensor("misc", [P, 136], F32, kind="ExternalInput")
    out_d = nc.dram_tensor("out", [P, 32], F32, kind="ExternalOutput")

    with tile.TileContext(nc) as tc:
        with tc.tile_pool(name="persist", bufs=1) as pp, \
             tc.tile_pool(name="acc_ps", bufs=1, space="PSUM") as accp:

            geom = pp.tile([GR, 2048], F16)
            accst = pp.tile([P, KC * ACC_CW], F16)
            misc = pp.tile([P, 136], F32)
            nc.sync.dma_start(geom[:], geom_d[:])
            nc.sync.dma_start(accst[:, 0 : 4 * ACC_CW], accst_d[:, 0 : 4 * ACC_CW])
            nc.sync.dma_start(accst[:, 4 * ACC_CW :], accst_d[:, 4 * ACC_CW :])
            nc.sync.dma_start(misc[:], misc_d[:])
            stat = geom[:, 0:1024]
            mov = geom[:, 1024:2048]
            dkk = misc[:, 0:8]
            tmi0 = misc[:, 8:24]
            tmi1 = misc[:, 24:40]
            EXPh = misc[:, 40:88]          # rows 0:21: Hm-entry -> 6x7 expander
            EXPr = misc[:, 88:136]         # rows 0:6: rhs-entry -> col 6

            lnbias = pp.tile([P, 1], F32)
            nc.vector.memset(lnbias[:], 1e-12)
            sallh = pp.tile([P, 2048], F16)   # ||e_i-e_k||^2, relu'd
            affh = pp.tile([P, 2048], F16)    # exp(-||e_i-e_k||)
            atmp = pp.tile([P, 2048], F32)
            dall = pp.tile([P, 2048], F32)    # d = 1/Zp
            dhall = pp.tile([P, 2048], F16)
            d2all = pp.tile([P, 2048], F16)
            dXall = pp.tile([P, 2048], F16)   # dX = Xp/Zp - Xkk
            dYall = pp.tile([P, 2048], F16)

            accH = accp.tile([21, SLAB], F32)
            accR = accp.tile([6, SLAB], F32)

            with tc.tile_pool(name="mm_ps", bufs=2, space="PSUM") as mmp, \
                 tc.tile_pool(name="work", bufs=2) as wp:

                def pass_a(c):
                    ck = slice(c * P, (c + 1) * P)
                    cs = slice(c * SLAB, (c + 1) * SLAB)
                    Zs = mmp.tile([P, 2 * SLAB], F32, name=f"Zs{c}", tag="Zs")
                    XY = mmp.tile([P, 2 * SLAB], F32, name=f"XY{c}", tag="XY")
                    nc.tensor.matmul(Zs[:], stat[:, ck], mov[:, 512:1024],
                                     start=True, stop=True)
                    nc.tensor.matmul(XY[:], stat[:, ck], mov[:, 0:512],
                                     start=True, stop=True)
                    d32 = dall[:, cs]
                    nc.vector.reciprocal_approx_fast(d32, Zs[:, 0:SLAB])
                    nc.scalar.activation(sallh[:, cs], Zs[:, SLAB : 2 * SLAB], AF.Relu)
                    nc.vector.tensor_tensor(dXall[:, cs], XY[:, 0:SLAB], d32, ALU.mult)
                    nc.vector.tensor_tensor(dYall[:, cs], XY[:, SLAB : 2 * SLAB], d32, ALU.mult)
                    nc.scalar.copy(dhall[:, cs], d32)
                    nc.scalar.square(d2all[:, cs], d32)

                def pass_b1(c):
                    # rhs bands (affinity-free): d, d2, ddX, ddY, ddX2, ddY2, d2dD
                    cs = slice(c * SLAB, (c + 1) * SLAB)
                    ro = c * ACC_CW + NHB * 21
                    dXt = dXall[:, cs]
                    dYt = dYall[:, cs]
                    dh = dhall[:, cs]
                    d2h = d2all[:, cs]

                    def wt(nm):
                        return wp.tile([P, SLAB], F16, name=f"{nm}{c}", tag=nm)

                    bdX = wt("bdX")
                    bdY = wt("bdY")
                    bd2dD = wt("bd2dD")
                    bdX2 = wt("bdX2")
                    bdY2 = wt("bdY2")
                    nc.vector.tensor_tensor(bdX[:], dh, dXt, ALU.mult)
                    nc.vector.tensor_tensor(bdY[:], dh, dYt, ALU.mult)
                    nc.vector.scalar_tensor_tensor(bd2dD[:], dall[:, cs],
                                                   dkk[:, c : c + 1],
                                                   d2h, ALU.subtract, ALU.mult)
                    nc.gpsimd.tensor_tensor(bdX2[:], bdX[:], dXt, ALU.mult)
                    nc.gpsimd.tensor_tensor(bdY2[:], bdY[:], dYt, ALU.mult)
                    rbands = [dh, d2h, bdX[:], bdY[:], bdX2[:], bdY2[:], bd2dD[:]]
                    for m, bt in enumerate(rbands):
                        nc.tensor.matmul(
                            accR[:], accst[:, ro + m * 6 : ro + (m + 1) * 6], bt,
                            start=(c == 0 and m == 0),
                            stop=(c == KC - 1 and m == NRB - 1))

                def aff_batch():
                    nc.scalar.activation(atmp[:], sallh[:], AF.Ln, bias=lnbias[:])
                    nc.scalar.activation(atmp[:], atmp[:], AF.Exp, scale=0.5)
                    nc.scalar.activation(affh[:], atmp[:], AF.Exp, scale=-1.0)

                def pass_b2(c):
                    # affinity-weighted Hm bands: A, AdX, AdY, AdX2, AdY2, Ad2
                    cs = slice(c * SLAB, (c + 1) * SLAB)
                    co = c * ACC_CW
                    dXt = dXall[:, cs]
                    dYt = dYall[:, cs]
                    d2h = d2all[:, cs]

                    def wt(nm):
                        return wp.tile([P, SLAB], F16, name=f"{nm}{c}", tag=nm)

                    bA = wt("bA")
                    bAdX = wt("bAdX")
                    bAdY = wt("bAdY")
                    bAdX2 = wt("bAdX2")
                    bAdY2 = wt("bAdY2")
                    bAd2 = wt("bAd2")
                    nc.vector.tensor_tensor(bA[:], affh[:, cs], d2h, ALU.mult)
                    nc.vector.tensor_tensor(bAdX[:], bA[:], dXt, ALU.mult)
                    nc.vector.tensor_tensor(bAdY[:], bA[:], dYt, ALU.mult)
                    nc.gpsimd.tensor_tensor(bAdX2[:], bAdX[:], dXt, ALU.mult)
                    nc.gpsimd.tensor_tensor(bAdY2[:], bAdY[:], dYt, ALU.mult)
                    nc.gpsimd.tensor_tensor(bAd2[:], bA[:], d2h, ALU.mult)
                    hbands = [bA[:], bAdX[:], bAdY[:], bAdX2[:], bAdY2[:], bAd2[:]]
                    for m, bt in enumerate(hbands):
                        nc.tensor.matmul(
                            accH[:], accst[:, co + m * 21 : co + (m + 1) * 21], bt,
                            start=(c == 0 and m == 0),
                            stop=(c == KC - 1 and m == NHB - 1))

                for c in range(KC):
                    pass_a(c)
                    pass_b1(c)
                aff_batch()
                for c in range(KC):
                    pass_b2(c)

            # ---------------- solve / exp map / compose -----------------
            with tc.tile_pool(name="post", bufs=2) as qp, \
                 tc.tile_pool(name="post_ps", bufs=2, space="PSUM") as qps:
                acc_sbH = qp.tile([21, SLAB], F32)
                acc_sbR = qp.tile([6, SLAB], F32)
                nc.scalar.copy(acc_sbH[:], accH[:])
                nc.scalar.copy(acc_sbR[:], accR[:])

                # expand 27 entry rows -> [anchor, 6x7 augmented] per half
                hb = qp.tile([P, 2 * NE], F32)  # ih-major: [0:48]=ih0, [48:96]=ih1
                for ih in range(2):
                    hb_ps = qps.tile([P, NE], F32, name=f"hbps{ih}", tag="hbps")
                    nc.tensor.matmul(hb_ps[:], acc_sbH[:, ih * P : (ih + 1) * P],
                                     EXPh[0:21, :], start=True, stop=False)
                    nc.tensor.matmul(hb_ps[:], acc_sbR[:, ih * P : (ih + 1) * P],
                                     EXPr[0:6, :], start=False, stop=True)
                    nc.scalar.copy(hb[:, ih * NE : (ih + 1) * NE], hb_ps[:])

                # ---------------- Gauss-Jordan (both halves packed) --------
                def hbv(sl):
                    return hb[:].rearrange("p (i e) -> p i e", i=2)[:, :, sl]
                piv = qp.tile([P, 2], F32)
                f12 = qp.tile([P, 12], F32)
                upd = qp.tile([P, 84], F32)
                f12v = f12[:].rearrange("p (i r) -> p i r", i=2)
                updv = upd[:].rearrange("p (i r c) -> p i r c", r=6, c=7)
                for j in range(6):
                    nc.vector.reciprocal(piv[:], hb[:, 8 * j : 2 * NE : NE])
                    nc.vector.tensor_tensor(
                        f12v, hbv(slice(j, 42, 7)),
                        piv[:].to_broadcast((P, 2, 6)), ALU.mult)
                    nc.vector.memset(f12[:, j : 12 : 6], 0.0)
                    nc.vector.tensor_tensor(
                        updv, f12v.to_broadcast((P, 2, 6, 7)),
                        hbv(slice(7 * j, 7 * j + 7)).unsqueeze(2).to_broadcast((P, 2, 6, 7)),
                        ALU.mult)
                    hview = hbv(slice(0, 42)).rearrange("p i (r c) -> p i r c", c=7)
                    nc.vector.tensor_tensor(hview, hview, updv, ALU.subtract)
                dinv = qp.tile([P, 12], F32)
                delta = qp.tile([P, 12], F32)
                dinvv = dinv[:].rearrange("p (i r) -> p i r", i=2)
                deltav = delta[:].rearrange("p (i r) -> p i r", i=2)
                nc.vector.reciprocal(dinvv, hbv(slice(0, 42, 8)))
                nc.vector.tensor_tensor(deltav, hbv(slice(6, 42, 7)), dinvv, ALU.mult)

                # ------------- exp map coefficients via Taylor in th^2 -----
                wsq = qp.tile([P, 6], F32)
                th2 = qp.tile([P, 2], F32)
                wv = deltav[:, :, 3:6]
                vb = deltav[:, :, 0:3]
                wsqv = wsq[:].rearrange("p (i r) -> p i r", i=2)
                nc.vector.tensor_tensor(wsqv, wv, wv, ALU.mult)
                nc.vector.tensor_reduce(th2[:], wsqv, AX.X, ALU.add)
                tu2 = qp.tile([P, 2], F32)
                tu3 = qp.tile([P, 2], F32)
                nc.vector.tensor_tensor(tu2[:], th2[:], th2[:], ALU.mult)
                nc.vector.tensor_tensor(tu3[:], tu2[:], th2[:], ALU.mult)
                abc = qp.tile([P, 6], F32)   # col = coeff(A,B,C)*2 + ih
                t6 = qp.tile([P, 6], F32)
                # A = sin(t)/t, B = (1-cos t)/t^2, C = (t - sin t)/t^3 series
                nc.vector.tensor_scalar(t6[:, 0:2], th2[:], -1.0 / 6.0, 1.0, ALU.mult, ALU.add)
                nc.vector.tensor_scalar(t6[:, 2:4], th2[:], -1.0 / 24.0, 0.5, ALU.mult, ALU.add)
                nc.vector.tensor_scalar(t6[:, 4:6], th2[:], -1.0 / 120.0, 1.0 / 6.0, ALU.mult, ALU.add)
                nc.vector.scalar_tensor_tensor(abc[:, 0:2], tu2[:], 1.0 / 120.0,
                                               t6[:, 0:2], ALU.mult, ALU.add)
                nc.vector.scalar_tensor_tensor(abc[:, 2:4], tu2[:], 1.0 / 720.0,
                                               t6[:, 2:4], ALU.mult, ALU.add)
                nc.vector.scalar_tensor_tensor(abc[:, 4:6], tu2[:], 1.0 / 5040.0,
                                               t6[:, 4:6], ALU.mult, ALU.add)
                nc.vector.scalar_tensor_tensor(abc[:, 0:2], tu3[:], -1.0 / 5040.0,
                                               abc[:, 0:2], ALU.mult, ALU.add)
                nc.vector.scalar_tensor_tensor(abc[:, 2:4], tu3[:], -1.0 / 40320.0,
                                               abc[:, 2:4], ALU.mult, ALU.add)
                nc.vector.scalar_tensor_tensor(abc[:, 4:6], tu3[:], -1.0 / 362880.0,
                                               abc[:, 4:6], ALU.mult, ALU.add)

                # ------- packed both-half R/V, translation, compose --------
                def iv(tile_ap, n):
                    return tile_ap.rearrange("p (i e) -> p i e", i=n)
                u3 = qp.tile([P, 6], F32)       # (ih, r): w_r^2 - th^2
                u3v = iv(u3[:], 2)
                nc.vector.tensor_tensor(
                    u3v, wsqv, th2[:].unsqueeze(2).to_broadcast((P, 2, 3)),
                    ALU.subtract)
                Aw = qp.tile([P, 6], F32)
                Bw = qp.tile([P, 6], F32)
                Cw = qp.tile([P, 6], F32)
                dB = qp.tile([P, 6], F32)
                dC = qp.tile([P, 6], F32)
                nc.vector.tensor_tensor(
                    iv(Aw[:], 2), wv,
                    abc[:, 0:2].unsqueeze(2).to_broadcast((P, 2, 3)), ALU.mult)
                nc.vector.tensor_tensor(
                    iv(Bw[:], 2), wv,
                    abc[:, 2:4].unsqueeze(2).to_broadcast((P, 2, 3)), ALU.mult)
                nc.vector.tensor_tensor(
                    iv(Cw[:], 2), wv,
                    abc[:, 4:6].unsqueeze(2).to_broadcast((P, 2, 3)), ALU.mult)
                nc.vector.tensor_tensor(
                    iv(dB[:], 2), u3v,
                    abc[:, 2:4].unsqueeze(2).to_broadcast((P, 2, 3)), ALU.mult)
                nc.vector.tensor_tensor(
                    iv(dC[:], 2), u3v,
                    abc[:, 4:6].unsqueeze(2).to_broadcast((P, 2, 3)), ALU.mult)

                def wcol(r):
                    return delta[:, 3 + r : 12 : 6]
                qb = qp.tile([P, 6], F32)   # q01,q02,q12 x (2 ih): col=q*2+ih
                cb = qp.tile([P, 6], F32)
                nc.vector.tensor_tensor(qb[:, 0:2], Bw[:, 0:6:3], wcol(1), ALU.mult)
                nc.vector.tensor_tensor(qb[:, 2:4], Bw[:, 0:6:3], wcol(2), ALU.mult)
                nc.vector.tensor_tensor(qb[:, 4:6], Bw[:, 1:6:3], wcol(2), ALU.mult)
                nc.vector.tensor_tensor(cb[:, 0:2], Cw[:, 0:6:3], wcol(1), ALU.mult)
                nc.vector.tensor_tensor(cb[:, 2:4], Cw[:, 0:6:3], wcol(2), ALU.mult)
                nc.vector.tensor_tensor(cb[:, 4:6], Cw[:, 1:6:3], wcol(2), ALU.mult)

                # Rt: [P,24], col = (4r+c)*2 + ih, c=3 holds the translation
                # Vt: [P,18], col = (3r+c)*2 + ih
                Rt = qp.tile([P, 24], F32)
                Vt = qp.tile([P, 18], F32)
                for M, st, hat, dgc, oc in ((Rt, 8, Aw, dB, qb), (Vt, 6, Bw, dC, cb)):
                    nc.vector.tensor_scalar(M[:, 0:2], dgc[:, 0:6:3], 1.0, None, ALU.add)
                    nc.vector.tensor_tensor(M[:, 2:4], oc[:, 0:2], hat[:, 2:6:3], ALU.subtract)
                    nc.vector.tensor_tensor(M[:, 4:6], oc[:, 2:4], hat[:, 1:6:3], ALU.add)
                    nc.vector.tensor_tensor(M[:, st : st + 2], oc[:, 0:2], hat[:, 2:6:3], ALU.add)
                    nc.vector.tensor_scalar(M[:, st + 2 : st + 4], dgc[:, 1:6:3], 1.0, None, ALU.add)
                    nc.vector.tensor_tensor(M[:, st + 4 : st + 6], oc[:, 4:6], hat[:, 0:6:3], ALU.subtract)
                    nc.vector.tensor_tensor(M[:, 2 * st : 2 * st + 2], oc[:, 2:4], hat[:, 1:6:3], ALU.subtract)
                    nc.vector.tensor_tensor(M[:, 2 * st + 2 : 2 * st + 4], oc[:, 4:6], hat[:, 0:6:3], ALU.add)
                    nc.vector.tensor_scalar(M[:, 2 * st + 4 : 2 * st + 6], dgc[:, 2:6:3], 1.0, None, ALU.add)

                # translation t = V @ v  into Rt cols (4r+3)*2+ih
                trall = qp.tile([P, 18], F32)   # (i, r, c)
                trv = trall[:].rearrange("p (i r c) -> p i r c", r=3, c=3)
                nc.vector.tensor_tensor(
                    trv, Vt[:].rearrange("p (r c i) -> p i r c", r=3, c=3),
                    vb.unsqueeze(2).to_broadcast((P, 2, 3, 3)), ALU.mult)
                tvb = qp.tile([P, 6], F32)      # col = r*2 + ih
                tvbv = tvb[:].rearrange("p (r i) -> p i r", r=3)
                nc.vector.tensor_reduce(tvbv, trv, AX.X, ALU.add)
                Rtv4 = Rt[:].rearrange("p (r c i) -> p i r c", r=3, c=4)
                nc.vector.tensor_copy(Rtv4[:, :, :, 3:4], tvbv.unsqueeze(3))

                # compose out = dT @ Tmat, packed [P, 32] (ih-major)
                tmib = qp.tile([P, 32], F32)
                nc.vector.tensor_copy(tmib[:, 0:16], tmi0)
                nc.vector.tensor_copy(tmib[:, 16:32], tmi1)
                Ob = qp.tile([P, 32], F32)
                prod = qp.tile([P, 32], F32)    # (i, tc, c)
                prodv = prod[:].rearrange("p (i t c) -> p i t c", t=4, c=4)
                # tmib viewed as (p, i, tc, c): col = i*16 + 4c + tc
                tmv4 = tmib[:].rearrange("p (i c t) -> p i t c", c=4, t=4)
                obv4 = Ob[:].rearrange("p (i t) -> p i t", i=2)
                for r in range(3):
                    rtv = Rtv4[:, :, r : r + 1, :].to_broadcast((P, 2, 4, 4))
                    nc.vector.tensor_tensor(prodv, tmv4, rtv, ALU.mult)
                    nc.vector.tensor_reduce(obv4[:, :, 4 * r : 4 * r + 4], prodv,
                                            AX.X, ALU.add)
                nc.vector.tensor_copy(obv4[:, :, 12:16],
                                      tmib[:].rearrange("p (i t) -> p i t", i=2)[:, :, 12:16])
                nc.sync.dma_start(out_d[:], Ob[:])

    nc.compile()
    return nc


def _q16(x):
    return np.asarray(x, np.float16).astype(np.float64)


def prep_inputs(embeddings, revisions, weights, depth, pix_T_camXs, Tmat):
    f6 = np.float64
    emb = _q16(np.asarray(embeddings, f6).reshape(B, C, N))
    rev = np.asarray(revisions, f6).reshape(B, 3, N)
    wgt = np.asarray(weights, f6).reshape(B, 3, N)
    dep = np.asarray(depth, f6).reshape(B, N)
    pix = np.asarray(pix_T_camXs, f6)
    tm = np.asarray(Tmat, f6).reshape(B, N, 16)

    ys, xs = np.meshgrid(np.arange(H, dtype=f6), np.arange(W, dtype=f6),
                         indexing="ij")
    u = xs.reshape(-1)
    v = ys.reshape(-1)

    in_maps = []
    per_batch = []
    for b in range(B):
        fx, fy, x0, y0 = pix[b, 0, 0], pix[b, 1, 1], pix[b, 0, 2], pix[b, 1, 2]
        z = _q16(dep[b])
        X = _q16((u - x0) * dep[b] / fx)
        Y = _q16((v - y0) * dep[b] / fy)
        T0 = tm[b].reshape(N, 4, 4)
        dR = _q16(T0[:, :3, :3] - np.eye(3))   # rotations are near identity
        R = np.eye(3) + dR
        t = _q16(T0[:, :3, 3])
        xyz = np.stack([X, Y, z], -1)
        TjXj = np.einsum("kpq,kq->kp", R, xyz) + t
        Xkk = TjXj[:, 0] / TjXj[:, 2]
        Ykk = TjXj[:, 1] / TjXj[:, 2]
        dkk = 1.0 / TjXj[:, 2]
        on, zn = np.ones(N), np.zeros(N)
        JT0 = np.stack([on, zn, zn, zn, -z, Y], -1)
        JT1 = np.stack([zn, on, zn, z, zn, -X], -1)
        JT2 = np.stack([zn, zn, on, -Y, X, zn], -1)
        G0 = JT0 - Xkk[:, None] * JT2
        G1 = JT1 - Ykk[:, None] * JT2
        w0, w1, w2 = wgt[b, 0], wgt[b, 1], wgt[b, 2]
        r0, r1, r2 = rev[b, 0], rev[b, 1], rev[b, 2]

        def outer(a, bb):
            return np.einsum("kp,kq->kpq", a, bb)

        P00 = outer(G0, G0)
        P11 = outer(G1, G1)
        P22 = outer(JT2, JT2)
        P02 = outer(G0, JT2) + outer(JT2, G0)
        P12 = outer(G1, JT2) + outer(JT2, G1)
        wfx = (w0 * fx * fx)[:, None, None]
        wfy = (w1 * fy * fy)[:, None, None]
        SH = [wfx * P00 + wfy * P11, -wfx * P02, -wfy * P12,
              wfx * P22, wfy * P22, w2[:, None, None] * P22]
        SR = [-(fx * r0)[:, None] * G0 - (fy * r1)[:, None] * G1,
              r2[:, None] * JT2,
              fx * fx * G0 + (fx * r0)[:, None] * JT2,
              fy * fy * G1 + (fy * r1)[:, None] * JT2,
              -fx * fx * JT2,
              -fy * fy * JT2,
              -JT2]
        lam = 1.0 / (fx * fx)
        # acc stationary [128, KC*ACC_CW]; partition p of chunk c is k=c*128+p
        accst = np.zeros((P, KC * ACC_CW), f6)
        for c in range(KC):
            ks = slice(c * P, (c + 1) * P)
            co = c * ACC_CW
            for m, S in enumerate(SH):
                for ei, (p_, q_) in enumerate(HTRI):
                    accst[:, co + m * 21 + ei] = S[ks, p_, q_] * lam
            ro = co + NHB * 21
            for m, V in enumerate(SR):
                accst[:, ro + m * 6 : ro + (m + 1) * 6] = V[ks] * lam
        # geometry stationary [32, N]
        stat = np.zeros((GR, N), f6)
        stat[0:16] = -2.0 * emb[b]
        stat[16], stat[17], stat[18], stat[19] = X, Y, z, 1.0
        stat[20:24] = _q16(Xkk[None] * stat[16:20])
        stat[24:28] = _q16(Ykk[None] * stat[16:20])
        stat[28] = _q16((emb[b] ** 2).sum(0))
        stat[29] = _q16(X - Xkk * z)
        stat[30] = _q16(Y - Ykk * z)
        stat[31] = z
        per_batch.append(dict(stat=stat, accst=accst, dkk=dkk,
                              emb=emb[b], dR=dR, t=t))

    cmbH = np.zeros((P, 48), np.float32)
    for ei, (p_, q_) in enumerate(HTRI):
        cmbH[ei, p_ * 7 + q_] = 1.0
        if p_ != q_:
            cmbH[ei, q_ * 7 + p_] = 1.0
    cmbR = np.zeros((P, 48), np.float32)
    for p_ in range(6):
        cmbR[p_, p_ * 7 + 6] = 1.0

    for core in range(NCORES):
        b = core // 4
        s0 = (core % 4) * SLAB
        pb = per_batch[b]
        dRs = pb["dR"][s0 : s0 + SLAB]
        ts = pb["t"][s0 : s0 + SLAB]
        # moving operand [32, 4*SLAB]: X' | Y' | Z | s blocks
        mov = np.zeros((GR, 4 * SLAB), f6)
        for blk, row in ((0, 0), (1, 1), (2, 2)):
            mov[16:19, blk * SLAB : (blk + 1) * SLAB] = dRs[:, row, :].T
            mov[19, blk * SLAB : (blk + 1) * SLAB] = ts[:, row]
        mov[20:23, 0:SLAB] = -dRs[:, 2, :].T
        mov[23, 0:SLAB] = -ts[:, 2]
        mov[24:27, SLAB : 2 * SLAB] = -dRs[:, 2, :].T
        mov[27, SLAB : 2 * SLAB] = -ts[:, 2]
        mov[29, 0:SLAB] = 1.0
        mov[30, SLAB : 2 * SLAB] = 1.0
        mov[31, 2 * SLAB : 3 * SLAB] = 1.0
        ei_ = pb["emb"][:, s0 : s0 + SLAB]
        mov[0:16, 3 * SLAB : 4 * SLAB] = ei_
        mov[19, 3 * SLAB : 4 * SLAB] = _q16((ei_ ** 2).sum(0))
        mov[28, 3 * SLAB : 4 * SLAB] = 1.0

        geom = np.concatenate([pb["stat"], mov], 1)

        misc = np.zeros((P, 136), np.float32)
        misc[:, 0:8] = pb["dkk"].reshape(KC, P).T
        tms = np.asarray(tm[b][s0 : s0 + SLAB], np.float32)
        misc[:, 8:24] = tms[0:P]
        misc[:, 24:40] = tms[P : 2 * P]
        misc[:, 40:88] = cmbH
        misc[:, 88:136] = cmbR

        in_maps.append({
            "geom": np.ascontiguousarray(geom, np.float16),
            "accst": np.ascontiguousarray(pb["accst"], np.float16),
            "misc": np.ascontiguousarray(misc),
        })
    return in_maps


def gather_output(results):
    full = np.empty((B, N, 16), dtype=np.float32)
    for core in range(NCORES):
        b = core // 4
        s0 = (core % 4) * SLAB
        out = results[core]["out"]
        full[b, s0 : s0 + P] = out[:, 0:16]
        full[b, s0 + P : s0 + SLAB] = out[:, 16:32]
    return full.reshape(B, H, W, 4, 4)


_NC_CACHE = {}


def kernel(**inputs):
    if "nc" not in _NC_CACHE:
        _NC_CACHE["nc"] = build_nc()
    nc = _NC_CACHE["nc"]
    in_maps = prep_inputs(**inputs)
    res = run_bass_kernel_spmd(nc, in_maps, core_ids=list(range(NCORES)))
    return gather_output(res.results)


# revision 22
# speedup vs baseline: 2.0335x; 1.1919x over previous
"""Dense SE(3) Gauss-Newton kernel for Trainium2, sharded over 8 NeuronCores.

Sharding: core owns batch b = core//4 and a 256-anchor slab of the i axis;
the k axis (1024) runs in 8 chunks of 128 on the partition dimension with
anchors on the free dimension.

Per (i,k) the kernel materializes 13 fp16 "moving bands" (powers of the
projected-point deltas d, dX, dY, optionally weighted by the embedding
affinity); the 6x6 normal equations + rhs are accumulated straight into
PSUM as 27 entry rows by matmuls against host-precomputed per-k fp16
coefficient tables (per-entry stationaries).  The residual is decomposed
around each point's self-projection (delta form) so every band is
cancellation-free and fp16-safe.  The geometry inputs are pre-quantized
to fp16 on the host and every host-side constant (self-projections,
coefficient tables) is derived from the quantized values, which makes a
single-pass fp16 geometry matmul exact-enough by construction.  1/Z runs
on the Vector engine (fast custom reciprocal); the affinity
exp(-sqrt(s)) is batched once so only two ACT table switches happen.
"""
import sys

sys.path.insert(0, "/opt/trn_rl_repo")

import numpy as np

from concourse import bacc, tile
import concourse.mybir as mybir
from concourse.bass_utils import run_bass_kernel_spmd

F32 = mybir.dt.float32
F16 = mybir.dt.float16
AF = mybir.ActivationFunctionType
ALU = mybir.AluOpType
AX = mybir.AxisListType

B, C, H, W = 2, 16, 32, 32
N = H * W
NCORES = 8
SLAB = 256
KC = 8
P = 128
GR = 32                       # geometry contraction rows
HTRI = [(p, q) for p in range(6) for q in range(p, 6)]  # 21 entries
NHB = 6                       # Hm bands: A, AdX, AdY, AdX2, AdY2, Ad2
NRB = 7                       # rhs bands: d, d2, ddX, ddY, ddX2, ddY2, d2dD
ACC_CW = NHB * 21 + NRB * 6   # 168 stationary cols per chunk
NE = 48                       # augmented 6x7 system padded


def build_nc():
    nc = bacc.Bacc("TRN2", target_bir_lowering=False, debug=False)

    geom_d = nc.dram_tensor("geom", [GR, 2048], F16, kind="ExternalInput")
    accst_d = nc.dram_tensor("accst", [P, KC * ACC_CW], F16, kind="ExternalInput")
    misc_d = nc.dram_tensor("misc", [P, 136], F32, kind="ExternalInput")
    out_d = nc.dram_tensor("out", [P, 32], F32, kind="ExternalOutput")

    with tile.TileContext(nc) as tc:
        with tc.tile_pool(name="persist", bufs=1) as pp, \
             tc.tile_pool(name="acc_ps", bufs=1, space="PSUM") as accp:

            geom = pp.tile([GR, 2048], F16)
            accst = pp.tile([P, KC * ACC_CW], F16)
            misc = pp.tile([P, 136], F32)
            nc.sync.dma_start(geom[:], geom_d[:])
            nc.sync.dma_start(accst[:, 0 : 4 * ACC_CW], accst_d[:, 0 : 4 * ACC_CW])
            nc.sync.dma_start(accst[:, 4 * ACC_CW :], accst_d[:, 4 * ACC_CW :])
            nc.sync.dma_start(misc[:], misc_d[:])
            stat = geom[:, 0:1024]
            mov = geom[:, 1024:2048]
            dkk = misc[:, 0:8]
            tmi0 = misc[:, 8:24]
            tmi1 = misc[:, 24:40]
            EXPh = misc[:, 40:88]          # rows 0:21: Hm-entry -> 6x7 expander
            EXPr = misc[:, 88:136]         # rows 0:6: rhs-entry -> col 6

            lnbias = pp.tile([P, 1], F32)
            nc.vector.memset(lnbias[:], 1e-12)
            sallh = pp.tile([P, 2048], F16)   # ||e_i-e_k||^2, relu'd
            affh = pp.tile([P, 2048], F16)    # exp(-||e_i-e_k||)
            atmp = pp.tile([P, 2048], F32)
            dall = pp.tile([P, 2048], F32)    # d = 1/Zp
            dhall = pp.tile([P, 2048], F16)
            d2all = pp.tile([P, 2048], F16)
            dXall = pp.tile([P, 2048], F16)   # dX = Xp/Zp - Xkk
            dYall = pp.tile([P, 2048], F16)

            accH = accp.tile([21, SLAB], F32)
            accR = accp.tile([6, SLAB], F32)

            with tc.tile_pool(name="mm_ps", bufs=2, space="PSUM") as mmp, \
                 tc.tile_pool(name="work", bufs=2) as wp:

                def pass_a(c):
                    ck = slice(c * P, (c + 1) * P)
                    cs = slice(c * SLAB, (c + 1) * SLAB)
                    Zs = mmp.tile([P, 2 * SLAB], F32, name=f"Zs{c}", tag="Zs")
                    XY = mmp.tile([P, 2 * SLAB], F32, name=f"XY{c}", tag="XY")
                    nc.tensor.matmul(Zs[:], stat[:, ck], mov[:, 512:1024],
                                     start=True, stop=True)
                    nc.tensor.matmul(XY[:], stat[:, ck], mov[:, 0:512],
                                     start=True, stop=True)
                    d32 = dall[:, cs]
                    nc.vector.reciprocal_approx_fast(d32, Zs[:, 0:SLAB])
                    nc.scalar.activation(sallh[:, cs], Zs[:, SLAB : 2 * SLAB], AF.Relu)
                    nc.vector.tensor_tensor(dXall[:, cs], XY[:, 0:SLAB], d32, ALU.mult)
                    nc.vector.tensor_tensor(dYall[:, cs], XY[:, SLAB : 2 * SLAB], d32, ALU.mult)
                    nc.scalar.copy(dhall[:, cs], d32)
                    nc.scalar.square(d2all[:, cs], d32)

                def pass_b1(c):
                    # rhs bands (affinity-free): d, d2, ddX, ddY, ddX2, ddY2, d2dD
                    cs = slice(c * SLAB, (c + 1) * SLAB)
                    ro = c * ACC_CW + NHB * 21
                    dXt = dXall[:, cs]
                    dYt = dYall[:, cs]
                    dh = dhall[:, cs]
                    d2h = d2all[:, cs]

                    def wt(nm):
                        return wp.tile([P, SLAB], F16, name=f"{nm}{c}", tag=nm)

                    bdX = wt("bdX")
                    bdY = wt("bdY")
                    bd2dD = wt("bd2dD")
                    bdX2 = wt("bdX2")
                    bdY2 = wt("bdY2")
                    nc.vector.tensor_tensor(bdX[:], dh, dXt, ALU.mult)
                    nc.vector.tensor_tensor(bdY[:], dh, dYt, ALU.mult)
                    nc.vector.scalar_tensor_tensor(bd2dD[:], dall[:, cs],
                                                   dkk[:, c : c + 1],
                                                   d2h, ALU.subtract, ALU.mult)
                    nc.gpsimd.tensor_tensor(bdX2[:], bdX[:], dXt, ALU.mult)
                    nc.gpsimd.tensor_tensor(bdY2[:], bdY[:], dYt, ALU.mult)
                    rbands = [dh, d2h, bdX[:], bdY[:], bdX2[:], bdY2[:], bd2dD[:]]
                    for m, bt in enumerate(rbands):
                        nc.tensor.matmul(
                            accR[:], accst[:, ro + m * 6 : ro + (m + 1) * 6], bt,
                            start=(c == 0 and m == 0),
                            stop=(c == KC - 1 and m == NRB - 1))

                def aff_batch(h):
                    hs = slice(h * 4 * SLAB, (h + 1) * 4 * SLAB)
                    at = atmp[:, hs]
                    nc.scalar.activation(at, sallh[:, hs], AF.Ln, bias=lnbias[:])
                    nc.scalar.activation(at, at, AF.Exp, scale=0.5)
                    nc.scalar.activation(affh[:, hs], at, AF.Exp, scale=-1.0)

                def pass_b2(c):
                    # affinity-weighted Hm bands: A, AdX, AdY, AdX2, AdY2, Ad2
                    cs = slice(c * SLAB, (c + 1) * SLAB)
                    co = c * ACC_CW
                    dXt = dXall[:, cs]
                    dYt = dYall[:, cs]
                    d2h = d2all[:, cs]

                    def wt(nm):
                        return wp.tile([P, SLAB], F16, name=f"{nm}{c}", tag=nm)

                    bA = wt("bA")
                    bAdX = wt("bAdX")
                    bAdY = wt("bAdY")
                    bAdX2 = wt("bAdX2")
                    bAdY2 = wt("bAdY2")
                    bAd2 = wt("bAd2")
                    nc.vector.tensor_tensor(bA[:], affh[:, cs], d2h, ALU.mult)
                    nc.vector.tensor_tensor(bAdX[:], bA[:], dXt, ALU.mult)
                    nc.vector.tensor_tensor(bAdY[:], bA[:], dYt, ALU.mult)
                    nc.vector.tensor_tensor(bAdX2[:], bAdX[:], dXt, ALU.mult)
                    nc.vector.tensor_tensor(bAdY2[:], bAdY[:], dYt, ALU.mult)
                    nc.gpsimd.tensor_tensor(bAd2[:], bA[:], d2h, ALU.mult)
                    hbands = [bA[:], bAdX[:], bAdY[:], bAdX2[:], bAdY2[:], bAd2[:]]
                    for m, bt in enumerate(hbands):
                        nc.tensor.matmul(
                            accH[:], accst[:, co + m * 21 : co + (m + 1) * 21], bt,
                            start=(c == 0 and m == 0),
                            stop=(c == KC - 1 and m == NHB - 1))

                for c in range(4):
                    pass_a(c)
                    pass_b1(c)
                aff_batch(0)
                for c in range(4, KC):
                    pass_a(c)
                    pass_b1(c)
                for c in range(4):
                    pass_b2(c)
                aff_batch(1)
                for c in range(4, KC):
                    pass_b2(c)

            # ---------------- solve / exp map / compose -----------------
            with tc.tile_pool(name="post", bufs=2) as qp, \
                 tc.tile_pool(name="post_ps", bufs=2, space="PSUM") as qps:
                acc_sbH = qp.tile([21, SLAB], F32)
                acc_sbR = qp.tile([6, SLAB], F32)
                nc.scalar.copy(acc_sbH[:], accH[:])
                nc.scalar.copy(acc_sbR[:], accR[:])

                # expand 27 entry rows -> [anchor, 6x7 augmented] per half
                hb = qp.tile([P, 2 * NE], F32)  # ih-major: [0:48]=ih0, [48:96]=ih1
                for ih in range(2):
                    hb_ps = qps.tile([P, NE], F32, name=f"hbps{ih}", tag="hbps")
                    nc.tensor.matmul(hb_ps[:], acc_sbH[:, ih * P : (ih + 1) * P],
                                     EXPh[0:21, :], start=True, stop=False)
                    nc.tensor.matmul(hb_ps[:], acc_sbR[:, ih * P : (ih + 1) * P],
                                     EXPr[0:6, :], start=False, stop=True)
                    nc.scalar.copy(hb[:, ih * NE : (ih + 1) * NE], hb_ps[:])

                # ---------------- Gauss-Jordan (both halves packed) --------
                def hbv(sl):
                    return hb[:].rearrange("p (i e) -> p i e", i=2)[:, :, sl]
                piv = qp.tile([P, 2], F32)
                f12 = qp.tile([P, 12], F32)
                upd = qp.tile([P, 84], F32)
                f12v = f12[:].rearrange("p (i r) -> p i r", i=2)
                updv = upd[:].rearrange("p (i r c) -> p i r c", r=6, c=7)
                for j in range(6):
                    nc.vector.reciprocal(piv[:], hb[:, 8 * j : 2 * NE : NE])
                    nc.vector.tensor_tensor(
                        f12v, hbv(slice(j, 42, 7)),
                        piv[:].to_broadcast((P, 2, 6)), ALU.mult)
                    nc.vector.memset(f12[:, j : 12 : 6], 0.0)
                    nc.vector.tensor_tensor(
                        updv, f12v.to_broadcast((P, 2, 6, 7)),
                        hbv(slice(7 * j, 7 * j + 7)).unsqueeze(2).to_broadcast((P, 2, 6, 7)),
                        ALU.mult)
                    hview = hbv(slice(0, 42)).rearrange("p i (r c) -> p i r c", c=7)
                    nc.vector.tensor_tensor(hview, hview, updv, ALU.subtract)
                dinv = qp.tile([P, 12], F32)
                delta = qp.tile([P, 12], F32)
                dinvv = dinv[:].rearrange("p (i r) -> p i r", i=2)
                deltav = delta[:].rearrange("p (i r) -> p i r", i=2)
                nc.vector.reciprocal(dinvv, hbv(slice(0, 42, 8)))
                nc.vector.tensor_tensor(deltav, hbv(slice(6, 42, 7)), dinvv, ALU.mult)

                # ------------- exp map coefficients via Taylor in th^2 -----
                wsq = qp.tile([P, 6], F32)
                th2 = qp.tile([P, 2], F32)
                wv = deltav[:, :, 3:6]
                vb = deltav[:, :, 0:3]
                wsqv = wsq[:].rearrange("p (i r) -> p i r", i=2)
                nc.vector.tensor_tensor(wsqv, wv, wv, ALU.mult)
                nc.vector.tensor_reduce(th2[:], wsqv, AX.X, ALU.add)
                tu2 = qp.tile([P, 2], F32)
                tu3 = qp.tile([P, 2], F32)
                nc.vector.tensor_tensor(tu2[:], th2[:], th2[:], ALU.mult)
                nc.vector.tensor_tensor(tu3[:], tu2[:], th2[:], ALU.mult)
                abc = qp.tile([P, 6], F32)   # col = coeff(A,B,C)*2 + ih
                t6 = qp.tile([P, 6], F32)
                # A = sin(t)/t, B = (1-cos t)/t^2, C = (t - sin t)/t^3 series
                nc.vector.tensor_scalar(t6[:, 0:2], th2[:], -1.0 / 6.0, 1.0, ALU.mult, ALU.add)
                nc.vector.tensor_scalar(t6[:, 2:4], th2[:], -1.0 / 24.0, 0.5, ALU.mult, ALU.add)
                nc.vector.tensor_scalar(t6[:, 4:6], th2[:], -1.0 / 120.0, 1.0 / 6.0, ALU.mult, ALU.add)
                nc.vector.scalar_tensor_tensor(abc[:, 0:2], tu2[:], 1.0 / 120.0,
                                               t6[:, 0:2], ALU.mult, ALU.add)
                nc.vector.scalar_tensor_tensor(abc[:, 2:4], tu2[:], 1.0 / 720.0,
                                               t6[:, 2:4], ALU.mult, ALU.add)
                nc.vector.scalar_tensor_tensor(abc[:, 4:6], tu2[:], 1.0 / 5040.0,
                                               t6[:, 4:6], ALU.mult, ALU.add)
                nc.vector.scalar_tensor_tensor(abc[:, 0:2], tu3[:], -1.0 / 5040.0,
                                               abc[:, 0:2], ALU.mult, ALU.add)
                nc.vector.scalar_tensor_tensor(abc[:, 2:4], tu3[:], -1.0 / 40320.0,
                                               abc[:, 2:4], ALU.mult, ALU.add)
                nc.vector.scalar_tensor_tensor(abc[:, 4:6], tu3[:], -1.0 / 362880.0,
                                               abc[:, 4:6], ALU.mult, ALU.add)

                # ------- packed both-half R/V, translation, compose --------
                def iv(tile_ap, n):
                    return tile_ap.rearrange("p (i e) -> p i e", i=n)
                u3 = qp.tile([P, 6], F32)       # (ih, r): w_r^2 - th^2
                u3v = iv(u3[:], 2)
                nc.vector.tensor_tensor(
                    u3v, wsqv, th2[:].unsqueeze(2).to_broadcast((P, 2, 3)),
                    ALU.subtract)
                Aw = qp.tile([P, 6], F32)
                Bw = qp.tile([P, 6], F32)
                Cw = qp.tile([P, 6], F32)
                dB = qp.tile([P, 6], F32)
                dC = qp.tile([P, 6], F32)
                nc.vector.tensor_tensor(
                    iv(Aw[:], 2), wv,
                    abc[:, 0:2].unsqueeze(2).to_broadcast((P, 2, 3)), ALU.mult)
                nc.vector.tensor_tensor(
                    iv(Bw[:], 2), wv,
                    abc[:, 2:4].unsqueeze(2).to_broadcast((P, 2, 3)), ALU.mult)
                nc.vector.tensor_tensor(
                    iv(Cw[:], 2), wv,
                    abc[:, 4:6].unsqueeze(2).to_broadcast((P, 2, 3)), ALU.mult)
                nc.vector.tensor_tensor(
                    iv(dB[:], 2), u3v,
                    abc[:, 2:4].unsqueeze(2).to_broadcast((P, 2, 3)), ALU.mult)
                nc.vector.tensor_tensor(
                    iv(dC[:], 2), u3v,
                    abc[:, 4:6].unsqueeze(2).to_broadcast((P, 2, 3)), ALU.mult)

                def wcol(r):
                    return delta[:, 3 + r : 12 : 6]
                qb = qp.tile([P, 6], F32)   # q01,q02,q12 x (2 ih): col=q*2+ih
                cb = qp.tile([P, 6], F32)
                nc.vector.tensor_tensor(qb[:, 0:2], Bw[:, 0:6:3], wcol(1), ALU.mult)
                nc.vector.tensor_tensor(qb[:, 2:4], Bw[:, 0:6:3], wcol(2), ALU.mult)
                nc.vector.tensor_tensor(qb[:, 4:6], Bw[:, 1:6:3], wcol(2), ALU.mult)
                nc.vector.tensor_tensor(cb[:, 0:2], Cw[:, 0:6:3], wcol(1), ALU.mult)
                nc.vector.tensor_tensor(cb[:, 2:4], Cw[:, 0:6:3], wcol(2), ALU.mult)
                nc.vector.tensor_tensor(cb[:, 4:6], Cw[:, 1:6:3], wcol(2), ALU.mult)

                # Rt: [P,24], col = (4r+c)*2 + ih, c=3 holds the translation
                # Vt: [P,18], col = (3r+c)*2 + ih
                Rt = qp.tile([P, 24], F32)
                Vt = qp.tile([P, 18], F32)
                for M, st, hat, dgc, oc in ((Rt, 8, Aw, dB, qb), (Vt, 6, Bw, dC, cb)):
                    nc.vector.tensor_scalar(M[:, 0:2], dgc[:, 0:6:3], 1.0, None, ALU.add)
                    nc.vector.tensor_tensor(M[:, 2:4], oc[:, 0:2], hat[:, 2:6:3], ALU.subtract)
                    nc.vector.tensor_tensor(M[:, 4:6], oc[:, 2:4], hat[:, 1:6:3], ALU.add)
                    nc.vector.tensor_tensor(M[:, st : st + 2], oc[:, 0:2], hat[:, 2:6:3], ALU.add)
                    nc.vector.tensor_scalar(M[:, st + 2 : st + 4], dgc[:, 1:6:3], 1.0, None, ALU.add)
                    nc.vector.tensor_tensor(M[:, st + 4 : st + 6], oc[:, 4:6], hat[:, 0:6:3], ALU.subtract)
                    nc.vector.tensor_tensor(M[:, 2 * st : 2 * st + 2], oc[:, 2:4], hat[:, 1:6:3], ALU.subtract)
                    nc.vector.tensor_tensor(M[:, 2 * st + 2 : 2 * st + 4], oc[:, 4:6], hat[:, 0:6:3], ALU.add)
                    nc.vector.tensor_scalar(M[:, 2 * st + 4 : 2 * st + 6], dgc[:, 2:6:3], 1.0, None, ALU.add)

                # translation t = V @ v  into Rt cols (4r+3)*2+ih
                trall = qp.tile([P, 18], F32)   # (i, r, c)
                trv = trall[:].rearrange("p (i r c) -> p i r c", r=3, c=3)
                nc.vector.tensor_tensor(
                    trv, Vt[:].rearrange("p (r c i) -> p i r c", r=3, c=3),
                    vb.unsqueeze(2).to_broadcast((P, 2, 3, 3)), ALU.mult)
                tvb = qp.tile([P, 6], F32)      # col = r*2 + ih
                tvbv = tvb[:].rearrange("p (r i) -> p i r", r=3)
                nc.vector.tensor_reduce(tvbv, trv, AX.X, ALU.add)
                Rtv4 = Rt[:].rearrange("p (r c i) -> p i r c", r=3, c=4)
                nc.vector.tensor_copy(Rtv4[:, :, :, 3:4], tvbv.unsqueeze(3))

                # compose out = dT @ Tmat, packed [P, 32] (ih-major)
                tmib = qp.tile([P, 32], F32)
                nc.vector.tensor_copy(tmib[:, 0:16], tmi0)
                nc.vector.tensor_copy(tmib[:, 16:32], tmi1)
                Ob = qp.tile([P, 32], F32)
                prod = qp.tile([P, 32], F32)    # (i, tc, c)
                prodv = prod[:].rearrange("p (i t c) -> p i t c", t=4, c=4)
                # tmib viewed as (p, i, tc, c): col = i*16 + 4c + tc
                tmv4 = tmib[:].rearrange("p (i c t) -> p i t c", c=4, t=4)
                obv4 = Ob[:].rearrange("p (i t) -> p i t", i=2)
                for r in range(3):
                    rtv = Rtv4[:, :, r : r + 1, :].to_broadcast((P, 2, 4, 4))
                    nc.vector.tensor_tensor(prodv, tmv4, rtv, ALU.mult)
                    nc.vector.tensor_reduce(obv4[:, :, 4 * r : 4 * r + 4], prodv,
                                            AX.X, ALU.add)
                nc.vector.tensor_copy(obv4[:, :, 12:16],
                                      tmib[:].rearrange("p (i t) -> p i t", i=2)[:, :, 12:16])
                nc.sync.dma_start(out_d[:], Ob[:])

    nc.compile()
    return nc


def _q16(x):
    return np.asarray(x, np.float16).astype(np.float64)


def prep_inputs(embeddings, revisions, weights, depth, pix_T_camXs, Tmat):
    f6 = np.float64
    emb = _q16(np.asarray(embeddings, f6).reshape(B, C, N))
    rev = np.asarray(revisions, f6).reshape(B, 3, N)
    wgt = np.asarray(weights, f6).reshape(B, 3, N)
    dep = np.asarray(depth, f6).reshape(B, N)
    pix = np.asarray(pix_T_camXs, f6)
    tm = np.asarray(Tmat, f6).reshape(B, N, 16)

    ys, xs = np.meshgrid(np.arange(H, dtype=f6), np.arange(W, dtype=f6),
                         indexing="ij")
    u = xs.reshape(-1)
    v = ys.reshape(-1)

    in_maps = []
    per_batch = []
    for b in range(B):
        fx, fy, x0, y0 = pix[b, 0, 0], pix[b, 1, 1], pix[b, 0, 2], pix[b, 1, 2]
        z = _q16(dep[b])
        X = _q16((u - x0) * dep[b] / fx)
        Y = _q16((v - y0) * dep[b] / fy)
        T0 = tm[b].reshape(N, 4, 4)
        dR = _q16(T0[:, :3, :3] - np.eye(3))   # rotations are near identity
        R = np.eye(3) + dR
        t = _q16(T0[:, :3, 3])
        xyz = np.stack([X, Y, z], -1)
        TjXj = np.einsum("kpq,kq->kp", R, xyz) + t
        Xkk = TjXj[:, 0] / TjXj[:, 2]
        Ykk = TjXj[:, 1] / TjXj[:, 2]
        dkk = 1.0 / TjXj[:, 2]
        on, zn = np.ones(N), np.zeros(N)
        JT0 = np.stack([on, zn, zn, zn, -z, Y], -1)
        JT1 = np.stack([zn, on, zn, z, zn, -X], -1)
        JT2 = np.stack([zn, zn, on, -Y, X, zn], -1)
        G0 = JT0 - Xkk[:, None] * JT2
        G1 = JT1 - Ykk[:, None] * JT2
        w0, w1, w2 = wgt[b, 0], wgt[b, 1], wgt[b, 2]
        r0, r1, r2 = rev[b, 0], rev[b, 1], rev[b, 2]

        def outer(a, bb):
            return np.einsum("kp,kq->kpq", a, bb)

        P00 = outer(G0, G0)
        P11 = outer(G1, G1)
        P22 = outer(JT2, JT2)
        P02 = outer(G0, JT2) + outer(JT2, G0)
        P12 = outer(G1, JT2) + outer(JT2, G1)
        wfx = (w0 * fx * fx)[:, None, None]
        wfy = (w1 * fy * fy)[:, None, None]
        SH = [wfx * P00 + wfy * P11, -wfx * P02, -wfy * P12,
              wfx * P22, wfy * P22, w2[:, None, None] * P22]
        SR = [-(fx * r0)[:, None] * G0 - (fy * r1)[:, None] * G1,
              r2[:, None] * JT2,
              fx * fx * G0 + (fx * r0)[:, None] * JT2,
              fy * fy * G1 + (fy * r1)[:, None] * JT2,
              -fx * fx * JT2,
              -fy * fy * JT2,
              -JT2]
        lam = 1.0 / (fx * fx)
        # acc stationary [128, KC*ACC_CW]; partition p of chunk c is k=c*128+p
        accst = np.zeros((P, KC * ACC_CW), f6)
        for c in range(KC):
            ks = slice(c * P, (c + 1) * P)
            co = c * ACC_CW
            for m, S in enumerate(SH):
                for ei, (p_, q_) in enumerate(HTRI):
                    accst[:, co + m * 21 + ei] = S[ks, p_, q_] * lam
            ro = co + NHB * 21
            for m, V in enumerate(SR):
                accst[:, ro + m * 6 : ro + (m + 1) * 6] = V[ks] * lam
        # geometry stationary [32, N]
        stat = np.zeros((GR, N), f6)
        stat[0:16] = -2.0 * emb[b]
        stat[16], stat[17], stat[18], stat[19] = X, Y, z, 1.0
        stat[20:24] = _q16(Xkk[None] * stat[16:20])
        stat[24:28] = _q16(Ykk[None] * stat[16:20])
        stat[28] = _q16((emb[b] ** 2).sum(0))
        stat[29] = _q16(X - Xkk * z)
        stat[30] = _q16(Y - Ykk * z)
        stat[31] = z
        per_batch.append(dict(stat=stat, accst=accst, dkk=dkk,
                              emb=emb[b], dR=dR, t=t))

    cmbH = np.zeros((P, 48), np.float32)
    for ei, (p_, q_) in enumerate(HTRI):
        cmbH[ei, p_ * 7 + q_] = 1.0
        if p_ != q_:
            cmbH[ei, q_ * 7 + p_] = 1.0
    cmbR = np.zeros((P, 48), np.float32)
    for p_ in range(6):
        cmbR[p_, p_ * 7 + 6] = 1.0

    for core in range(NCORES):
        b = core // 4
        s0 = (core % 4) * SLAB
        pb = per_batch[b]
        dRs = pb["dR"][s0 : s0 + SLAB]
        ts = pb["t"][s0 : s0 + SLAB]
        # moving operand [32, 4*SLAB]: X' | Y' | Z | s blocks
        mov = np.zeros((GR, 4 * SLAB), f6)
        for blk, row in ((0, 0), (1, 1), (2, 2)):
            mov[16:19, blk * SLAB : (blk + 1) * SLAB] = dRs[:, row, :].T
            mov[19, blk * SLAB : (blk + 1) * SLAB] = ts[:, row]
        mov[20:23, 0:SLAB] = -dRs[:, 2, :].T
        mov[23, 0:SLAB] = -ts[:, 2]
        mov[24:27, SLAB : 2 * SLAB] = -dRs[:, 2, :].T
        mov[27, SLAB : 2 * SLAB] = -ts[:, 2]
        mov[29, 0:SLAB] = 1.0
        mov[30, SLAB : 2 * SLAB] = 1.0
        mov[31, 2 * SLAB : 3 * SLAB] = 1.0
        ei_ = pb["emb"][:, s0 : s0 + SLAB]
        mov[0:16, 3 * SLAB : 4 * SLAB] = ei_
        mov[19, 3 * SLAB : 4 * SLAB] = _q16((ei_ ** 2).sum(0))
        mov[28, 3 * SLAB : 4 * SLAB] = 1.0

        geom = np.concatenate([pb["stat"], mov], 1)

        misc = np.zeros((P, 136), np.float32)
        misc[:, 0:8] = pb["dkk"].reshape(KC, P).T
        tms = np.asarray(tm[b][s0 : s0 + SLAB], np.float32)
        misc[:, 8:24] = tms[0:P]
        misc[:, 24:40] = tms[P : 2 * P]
        misc[:, 40:88] = cmbH
        misc[:, 88:136] = cmbR

        in_maps.append({
            "geom": np.ascontiguousarray(geom, np.float16),
            "accst": np.ascontiguousarray(pb["accst"], np.float16),
            "misc": np.ascontiguousarray(misc),
        })
    return in_maps


def gather_output(results):
    full = np.empty((B, N, 16), dtype=np.float32)
    for core in range(NCORES):
        b = core // 4
        s0 = (core % 4) * SLAB
        out = results[core]["out"]
        full[b, s0 : s0 + P] = out[:, 0:16]
        full[b, s0 + P : s0 + SLAB] = out[:, 16:32]
    return full.reshape(B, H, W, 4, 4)


_NC_CACHE = {}


def kernel(**inputs):
    if "nc" not in _NC_CACHE:
        _NC_CACHE["nc"] = build_nc()
    nc = _NC_CACHE["nc"]
    in_maps = prep_inputs(**inputs)
    res = run_bass_kernel_spmd(nc, in_maps, core_ids=list(range(NCORES)))
    return gather_output(res.results)


# revision 26
# speedup vs baseline: 2.0596x; 1.0129x over previous
"""Dense SE(3) Gauss-Newton kernel for Trainium2, sharded over 8 NeuronCores.

Sharding: core owns batch b = core//4 and a 256-anchor slab of the i axis;
the k axis (1024) runs in 8 chunks of 128 on the partition dimension with
anchors on the free dimension.

Per (i,k) the kernel materializes 13 fp16 "moving bands" (powers of the
projected-point deltas d, dX, dY, optionally weighted by the embedding
affinity); the 6x6 normal equations + rhs are accumulated straight into
PSUM as 27 entry rows by matmuls against host-precomputed per-k fp16
coefficient tables (per-entry stationaries).  The residual is decomposed
around each point's self-projection (delta form) so every band is
cancellation-free and fp16-safe.  The geometry inputs are pre-quantized
to fp16 on the host and every host-side constant (self-projections,
coefficient tables) is derived from the quantized values, which makes a
single-pass fp16 geometry matmul exact-enough by construction.  1/Z runs
on the Vector engine (fast custom reciprocal); the affinity
exp(-sqrt(s)) is batched once so only two ACT table switches happen.
"""
import sys

sys.path.insert(0, "/opt/trn_rl_repo")

import numpy as np

from concourse import bacc, tile
import concourse.mybir as mybir
from concourse.bass_utils import run_bass_kernel_spmd

F32 = mybir.dt.float32
F16 = mybir.dt.float16
AF = mybir.ActivationFunctionType
ALU = mybir.AluOpType
AX = mybir.AxisListType

B, C, H, W = 2, 16, 32, 32
N = H * W
NCORES = 8
SLAB = 256
KC = 8
P = 128
GR = 32                       # geometry contraction rows
HTRI = [(p, q) for p in range(6) for q in range(p, 6)]  # 21 entries
NHB = 6                       # Hm bands: A, AdX, AdY, AdX2, AdY2, Ad2
NRB = 7                       # rhs bands: d, d2, ddX, ddY, ddX2, ddY2, d2dD
ACC_CW = NHB * 21 + NRB * 6   # 168 stationary cols per chunk
NE = 48                       # augmented 6x7 system padded


def build_nc():
    nc = bacc.Bacc("TRN2", target_bir_lowering=False, debug=False)

    geom_d = nc.dram_tensor("geom", [GR, 2048], F16, kind="ExternalInput")
    accst_d = nc.dram_tensor("accst", [P, KC * ACC_CW], F16, kind="ExternalInput")
    misc_d = nc.dram_tensor("misc", [P, 136], F32, kind="ExternalInput")
    out_d = nc.dram_tensor("out", [P, 32], F32, kind="ExternalOutput")

    with tile.TileContext(nc) as tc:
        with tc.tile_pool(name="persist", bufs=1) as pp, \
             tc.tile_pool(name="acc_ps", bufs=1, space="PSUM") as accp:

            geom = pp.tile([GR, 2048], F16)
            accst = pp.tile([P, KC * ACC_CW], F16)
            misc = pp.tile([P, 136], F32)
            nc.sync.dma_start(geom[:], geom_d[:])
            nc.sync.dma_start(accst[:, 0 : 4 * ACC_CW], accst_d[:, 0 : 4 * ACC_CW])
            nc.sync.dma_start(accst[:, 4 * ACC_CW :], accst_d[:, 4 * ACC_CW :])
            nc.sync.dma_start(misc[:], misc_d[:])
            stat = geom[:, 0:1024]
            mov = geom[:, 1024:2048]
            dkk = misc[:, 0:8]
            tmi0 = misc[:, 8:24]
            tmi1 = misc[:, 24:40]
            EXPh = misc[:, 40:88]          # rows 0:21: Hm-entry -> 6x7 expander
            EXPr = misc[:, 88:136]         # rows 0:6: rhs-entry -> col 6

            lnbias = pp.tile([P, 1], F32)
            nc.vector.memset(lnbias[:], 1e-12)
            sallh = pp.tile([P, 2048], F16)   # ||e_i-e_k||^2, relu'd
            affh = pp.tile([P, 2048], F16)    # exp(-||e_i-e_k||)
            atmp = pp.tile([P, 2048], F32)
            dall = pp.tile([P, 2048], F32)    # d = 1/Zp
            dhall = pp.tile([P, 2048], F16)
            d2all = pp.tile([P, 2048], F16)
            dXall = pp.tile([P, 2048], F16)   # dX = Xp/Zp - Xkk
            dYall = pp.tile([P, 2048], F16)

            accH = accp.tile([21, SLAB], F32)
            accR = accp.tile([6, SLAB], F32)

            with tc.tile_pool(name="mm_ps", bufs=2, space="PSUM") as mmp, \
                 tc.tile_pool(name="work", bufs=3) as wp:

                def pass_a(c):
                    ck = slice(c * P, (c + 1) * P)
                    cs = slice(c * SLAB, (c + 1) * SLAB)
                    Zs = mmp.tile([P, 2 * SLAB], F32, name=f"Zs{c}", tag="Zs")
                    XY = mmp.tile([P, 2 * SLAB], F32, name=f"XY{c}", tag="XY")
                    nc.tensor.matmul(Zs[:], stat[:, ck], mov[:, 512:1024],
                                     start=True, stop=True)
                    nc.tensor.matmul(XY[:], stat[:, ck], mov[:, 0:512],
                                     start=True, stop=True)
                    d32 = dall[:, cs]
                    nc.vector.reciprocal_approx_fast(d32, Zs[:, 0:SLAB])
                    nc.scalar.activation(sallh[:, cs], Zs[:, SLAB : 2 * SLAB], AF.Relu)
                    nc.vector.tensor_tensor(dXall[:, cs], XY[:, 0:SLAB], d32, ALU.mult)
                    nc.vector.tensor_tensor(dYall[:, cs], XY[:, SLAB : 2 * SLAB], d32, ALU.mult)
                    nc.scalar.copy(dhall[:, cs], d32)
                    nc.scalar.square(d2all[:, cs], d32)

                def pass_b1(c):
                    # rhs bands (affinity-free): d, d2, ddX, ddY, ddX2, ddY2, d2dD
                    cs = slice(c * SLAB, (c + 1) * SLAB)
                    ro = c * ACC_CW + NHB * 21
                    dXt = dXall[:, cs]
                    dYt = dYall[:, cs]
                    dh = dhall[:, cs]
                    d2h = d2all[:, cs]

                    def wt(nm):
                        return wp.tile([P, SLAB], F16, name=f"{nm}{c}", tag=nm)

                    bdX = wt("bdX")
                    bdY = wt("bdY")
                    bd2dD = wt("bd2dD")
                    bdX2 = wt("bdX2")
                    bdY2 = wt("bdY2")
                    nc.vector.tensor_tensor(bdX[:], dh, dXt, ALU.mult)
                    nc.vector.tensor_tensor(bdY[:], dh, dYt, ALU.mult)
                    nc.gpsimd.tensor_tensor(bdX2[:], bdX[:], dXt, ALU.mult)
                    nc.gpsimd.tensor_tensor(bdY2[:], bdY[:], dYt, ALU.mult)
                    nc.vector.scalar_tensor_tensor(bd2dD[:], dall[:, cs],
                                                   dkk[:, c : c + 1],
                                                   d2h, ALU.subtract, ALU.mult)
                    # matmuls in band-availability order; m indexes the
                    # stationary layout [d, d2, ddX, ddY, ddX2, ddY2, d2dD]
                    rbands = [(dh, 0), (d2h, 1), (bdX[:], 2), (bdY[:], 3),
                              (bd2dD[:], 6), (bdX2[:], 4), (bdY2[:], 5)]
                    for j, (bt, m) in enumerate(rbands):
                        nc.tensor.matmul(
                            accR[:], accst[:, ro + m * 6 : ro + (m + 1) * 6], bt,
                            start=(c == 0 and j == 0),
                            stop=(c == KC - 1 and j == NRB - 1))

                def aff_batch(h):
                    hs = slice(h * 4 * SLAB, (h + 1) * 4 * SLAB)
                    at = atmp[:, hs]
                    nc.scalar.activation(at, sallh[:, hs], AF.Ln, bias=lnbias[:])
                    nc.scalar.activation(at, at, AF.Exp, scale=0.5)
                    nc.scalar.activation(affh[:, hs], at, AF.Exp, scale=-1.0)

                def pass_b2(c):
                    # affinity-weighted Hm bands: A, AdX, AdY, AdX2, AdY2, Ad2
                    cs = slice(c * SLAB, (c + 1) * SLAB)
                    co = c * ACC_CW
                    dXt = dXall[:, cs]
                    dYt = dYall[:, cs]
                    d2h = d2all[:, cs]

                    def wt(nm):
                        return wp.tile([P, SLAB], F16, name=f"{nm}{c}", tag=nm)

                    bA = wt("bA")
                    bAdX = wt("bAdX")
                    bAdY = wt("bAdY")
                    bAdX2 = wt("bAdX2")
                    bAdY2 = wt("bAdY2")
                    bAd2 = wt("bAd2")
                    nc.vector.tensor_tensor(bA[:], affh[:, cs], d2h, ALU.mult)
                    nc.gpsimd.tensor_tensor(bAd2[:], bA[:], d2h, ALU.mult)
                    nc.vector.tensor_tensor(bAdX[:], bA[:], dXt, ALU.mult)
                    nc.vector.tensor_tensor(bAdY[:], bA[:], dYt, ALU.mult)
                    nc.vector.tensor_tensor(bAdX2[:], bAdX[:], dXt, ALU.mult)
                    nc.vector.tensor_tensor(bAdY2[:], bAdY[:], dYt, ALU.mult)
                    hbands = [bA[:], bAdX[:], bAdY[:], bAdX2[:], bAdY2[:], bAd2[:]]
                    for m, bt in enumerate(hbands):
                        nc.tensor.matmul(
                            accH[:], accst[:, co + m * 21 : co + (m + 1) * 21], bt,
                            start=(c == 0 and m == 0),
                            stop=(c == KC - 1 and m == NHB - 1))

                # pass_a runs one chunk ahead so the PE queue always has
                # geometry matmuls to chew on while bands are produced
                pass_a(0)
                for c in range(1, 4):
                    pass_a(c)
                    pass_b1(c - 1)
                aff_batch(0)
                for c in range(4, KC):
                    pass_a(c)
                    pass_b1(c - 1)
                pass_b1(KC - 1)
                for c in range(4):
                    pass_b2(c)
                aff_batch(1)
                for c in range(4, KC):
                    pass_b2(c)

            # ---------------- solve / exp map / compose -----------------
            with tc.tile_pool(name="post", bufs=2) as qp, \
                 tc.tile_pool(name="post_ps", bufs=2, space="PSUM") as qps:
                acc_sbH = qp.tile([21, SLAB], F32)
                acc_sbR = qp.tile([6, SLAB], F32)
                nc.scalar.copy(acc_sbH[:], accH[:])
                nc.scalar.copy(acc_sbR[:], accR[:])

                # expand 27 entry rows -> [anchor, 6x7 augmented] per half
                hb = qp.tile([P, 2 * NE], F32)  # ih-major: [0:48]=ih0, [48:96]=ih1
                for ih in range(2):
                    hb_ps = qps.tile([P, NE], F32, name=f"hbps{ih}", tag="hbps")
                    nc.tensor.matmul(hb_ps[:], acc_sbH[:, ih * P : (ih + 1) * P],
                                     EXPh[0:21, :], start=True, stop=False)
                    nc.tensor.matmul(hb_ps[:], acc_sbR[:, ih * P : (ih + 1) * P],
                                     EXPr[0:6, :], start=False, stop=True)
                    nc.scalar.copy(hb[:, ih * NE : (ih + 1) * NE], hb_ps[:])

                # ---------------- Gauss-Jordan (both halves packed) --------
                def hbv(sl):
                    return hb[:].rearrange("p (i e) -> p i e", i=2)[:, :, sl]
                piv = qp.tile([P, 2], F32)
                f12 = qp.tile([P, 12], F32)
                upd = qp.tile([P, 84], F32)
                f12v = f12[:].rearrange("p (i r) -> p i r", i=2)
                updv = upd[:].rearrange("p (i r c) -> p i r c", r=6, c=7)
                for j in range(6):
                    nc.vector.reciprocal(piv[:], hb[:, 8 * j : 2 * NE : NE])
                    nc.vector.tensor_tensor(
                        f12v, hbv(slice(j, 42, 7)),
                        piv[:].to_broadcast((P, 2, 6)), ALU.mult)
                    nc.vector.memset(f12[:, j : 12 : 6], 0.0)
                    nc.vector.tensor_tensor(
                        updv, f12v.to_broadcast((P, 2, 6, 7)),
                        hbv(slice(7 * j, 7 * j + 7)).unsqueeze(2).to_broadcast((P, 2, 6, 7)),
                        ALU.mult)
                    hview = hbv(slice(0, 42)).rearrange("p i (r c) -> p i r c", c=7)
                    nc.vector.tensor_tensor(hview, hview, updv, ALU.subtract)
                dinv = qp.tile([P, 12], F32)
                delta = qp.tile([P, 12], F32)
                dinvv = dinv[:].rearrange("p (i r) -> p i r", i=2)
                deltav = delta[:].rearrange("p (i r) -> p i r", i=2)
                nc.vector.reciprocal(dinvv, hbv(slice(0, 42, 8)))
                nc.vector.tensor_tensor(deltav, hbv(slice(6, 42, 7)), dinvv, ALU.mult)

                # ------------- exp map coefficients via Taylor in th^2 -----
                wsq = qp.tile([P, 6], F32)
                th2 = qp.tile([P, 2], F32)
                wv = deltav[:, :, 3:6]
                vb = deltav[:, :, 0:3]
                wsqv = wsq[:].rearrange("p (i r) -> p i r", i=2)
                nc.vector.tensor_tensor(wsqv, wv, wv, ALU.mult)
                nc.vector.tensor_reduce(th2[:], wsqv, AX.X, ALU.add)
                tu2 = qp.tile([P, 2], F32)
                tu3 = qp.tile([P, 2], F32)
                nc.vector.tensor_tensor(tu2[:], th2[:], th2[:], ALU.mult)
                nc.vector.tensor_tensor(tu3[:], tu2[:], th2[:], ALU.mult)
                abc = qp.tile([P, 6], F32)   # col = coeff(A,B,C)*2 + ih
                t6 = qp.tile([P, 6], F32)
                # A = sin(t)/t, B = (1-cos t)/t^2, C = (t - sin t)/t^3 series
                nc.vector.tensor_scalar(t6[:, 0:2], th2[:], -1.0 / 6.0, 1.0, ALU.mult, ALU.add)
                nc.vector.tensor_scalar(t6[:, 2:4], th2[:], -1.0 / 24.0, 0.5, ALU.mult, ALU.add)
                nc.vector.tensor_scalar(t6[:, 4:6], th2[:], -1.0 / 120.0, 1.0 / 6.0, ALU.mult, ALU.add)
                nc.vector.scalar_tensor_tensor(abc[:, 0:2], tu2[:], 1.0 / 120.0,
                                               t6[:, 0:2], ALU.mult, ALU.add)
                nc.vector.scalar_tensor_tensor(abc[:, 2:4], tu2[:], 1.0 / 720.0,
                                               t6[:, 2:4], ALU.mult, ALU.add)
                nc.vector.scalar_tensor_tensor(abc[:, 4:6], tu2[:], 1.0 / 5040.0,
                                               t6[:, 4:6], ALU.mult, ALU.add)
                nc.vector.scalar_tensor_tensor(abc[:, 0:2], tu3[:], -1.0 / 5040.0,
                                               abc[:, 0:2], ALU.mult, ALU.add)
                nc.vector.scalar_tensor_tensor(abc[:, 2:4], tu3[:], -1.0 / 40320.0,
                                               abc[:, 2:4], ALU.mult, ALU.add)
                nc.vector.scalar_tensor_tensor(abc[:, 4:6], tu3[:], -1.0 / 362880.0,
                                               abc[:, 4:6], ALU.mult, ALU.add)

                # ------- packed both-half R/V, translation, compose --------
                def iv(tile_ap, n):
                    return tile_ap.rearrange("p (i e) -> p i e", i=n)
                u3 = qp.tile([P, 6], F32)       # (ih, r): w_r^2 - th^2
                u3v = iv(u3[:], 2)
                nc.vector.tensor_tensor(
                    u3v, wsqv, th2[:].unsqueeze(2).to_broadcast((P, 2, 3)),
                    ALU.subtract)
                Aw = qp.tile([P, 6], F32)
                Bw = qp.tile([P, 6], F32)
                Cw = qp.tile([P, 6], F32)
                dB = qp.tile([P, 6], F32)
                dC = qp.tile([P, 6], F32)
                nc.vector.tensor_tensor(
                    iv(Aw[:], 2), wv,
                    abc[:, 0:2].unsqueeze(2).to_broadcast((P, 2, 3)), ALU.mult)
                nc.vector.tensor_tensor(
                    iv(Bw[:], 2), wv,
                    abc[:, 2:4].unsqueeze(2).to_broadcast((P, 2, 3)), ALU.mult)
                nc.vector.tensor_tensor(
                    iv(Cw[:], 2), wv,
                    abc[:, 4:6].unsqueeze(2).to_broadcast((P, 2, 3)), ALU.mult)
                nc.vector.tensor_tensor(
                    iv(dB[:], 2), u3v,
                    abc[:, 2:4].unsqueeze(2).to_broadcast((P, 2, 3)), ALU.mult)
                nc.vector.tensor_tensor(
                    iv(dC[:], 2), u3v,
                    abc[:, 4:6].unsqueeze(2).to_broadcast((P, 2, 3)), ALU.mult)

                def wcol(r):
                    return delta[:, 3 + r : 12 : 6]
                qb = qp.tile([P, 6], F32)   # q01,q02,q12 x (2 ih): col=q*2+ih
                cb = qp.tile([P, 6], F32)
                nc.vector.tensor_tensor(qb[:, 0:2], Bw[:, 0:6:3], wcol(1), ALU.mult)
                nc.vector.tensor_tensor(qb[:, 2:4], Bw[:, 0:6:3], wcol(2), ALU.mult)
                nc.vector.tensor_tensor(qb[:, 4:6], Bw[:, 1:6:3], wcol(2), ALU.mult)
                nc.vector.tensor_tensor(cb[:, 0:2], Cw[:, 0:6:3], wcol(1), ALU.mult)
                nc.vector.tensor_tensor(cb[:, 2:4], Cw[:, 0:6:3], wcol(2), ALU.mult)
                nc.vector.tensor_tensor(cb[:, 4:6], Cw[:, 1:6:3], wcol(2), ALU.mult)

                # Rt: [P,24], col = (4r+c)*2 + ih, c=3 holds the translation
                # Vt: [P,18], col = (3r+c)*2 + ih
                Rt = qp.tile([P, 24], F32)
                Vt = qp.tile([P, 18], F32)
                for M, st, hat, dgc, oc in ((Rt, 8, Aw, dB, qb), (Vt, 6, Bw, dC, cb)):
                    nc.vector.tensor_scalar(M[:, 0:2], dgc[:, 0:6:3], 1.0, None, ALU.add)
                    nc.vector.tensor_tensor(M[:, 2:4], oc[:, 0:2], hat[:, 2:6:3], ALU.subtract)
                    nc.vector.tensor_tensor(M[:, 4:6], oc[:, 2:4], hat[:, 1:6:3], ALU.add)
                    nc.vector.tensor_tensor(M[:, st : st + 2], oc[:, 0:2], hat[:, 2:6:3], ALU.add)
                    nc.vector.tensor_scalar(M[:, st + 2 : st + 4], dgc[:, 1:6:3], 1.0, None, ALU.add)
                    nc.vector.tensor_tensor(M[:, st + 4 : st + 6], oc[:, 4:6], hat[:, 0:6:3], ALU.subtract)
                    nc.vector.tensor_tensor(M[:, 2 * st : 2 * st + 2], oc[:, 2:4], hat[:, 1:6:3], ALU.subtract)
                    nc.vector.tensor_tensor(M[:, 2 * st + 2 : 2 * st + 4], oc[:, 4:6], hat[:, 0:6:3], ALU.add)
                    nc.vector.tensor_scalar(M[:, 2 * st + 4 : 2 * st + 6], dgc[:, 2:6:3], 1.0, None, ALU.add)

                # translation t = V @ v  into Rt cols (4r+3)*2+ih
                trall = qp.tile([P, 18], F32)   # (i, r, c)
                trv = trall[:].rearrange("p (i r c) -> p i r c", r=3, c=3)
                nc.vector.tensor_tensor(
                    trv, Vt[:].rearrange("p (r c i) -> p i r c", r=3, c=3),
                    vb.unsqueeze(2).to_broadcast((P, 2, 3, 3)), ALU.mult)
                tvb = qp.tile([P, 6], F32)      # col = r*2 + ih
                tvbv = tvb[:].rearrange("p (r i) -> p i r", r=3)
                nc.vector.tensor_reduce(tvbv, trv, AX.X, ALU.add)
                Rtv4 = Rt[:].rearrange("p (r c i) -> p i r c", r=3, c=4)
                nc.vector.tensor_copy(Rtv4[:, :, :, 3:4], tvbv.unsqueeze(3))

                # compose out = dT @ Tmat, packed [P, 32] (ih-major)
                tmib = qp.tile([P, 32], F32)
                nc.vector.tensor_copy(tmib[:, 0:16], tmi0)
                nc.vector.tensor_copy(tmib[:, 16:32], tmi1)
                Ob = qp.tile([P, 32], F32)
                prod = qp.tile([P, 32], F32)    # (i, tc, c)
                prodv = prod[:].rearrange("p (i t c) -> p i t c", t=4, c=4)
                # tmib viewed as (p, i, tc, c): col = i*16 + 4c + tc
                tmv4 = tmib[:].rearrange("p (i c t) -> p i t c", c=4, t=4)
                obv4 = Ob[:].rearrange("p (i t) -> p i t", i=2)
                for r in range(3):
                    rtv = Rtv4[:, :, r : r + 1, :].to_broadcast((P, 2, 4, 4))
                    nc.vector.tensor_tensor(prodv, tmv4, rtv, ALU.mult)
                    nc.vector.tensor_reduce(obv4[:, :, 4 * r : 4 * r + 4], prodv,
                                            AX.X, ALU.add)
                nc.vector.tensor_copy(obv4[:, :, 12:16],
                                      tmib[:].rearrange("p (i t) -> p i t", i=2)[:, :, 12:16])
                nc.sync.dma_start(out_d[:], Ob[:])

    nc.compile()
    return nc


def _q16(x):
    return np.asarray(x, np.float16).astype(np.float64)


def prep_inputs(embeddings, revisions, weights, depth, pix_T_camXs, Tmat):
    f6 = np.float64
    emb = _q16(np.asarray(embeddings, f6).reshape(B, C, N))
    rev = np.asarray(revisions, f6).reshape(B, 3, N)
    wgt = np.asarray(weights, f6).reshape(B, 3, N)
    dep = np.asarray(depth, f6).reshape(B, N)
    pix = np.asarray(pix_T_camXs, f6)
    tm = np.asarray(Tmat, f6).reshape(B, N, 16)

    ys, xs = np.meshgrid(np.arange(H, dtype=f6), np.arange(W, dtype=f6),
                         indexing="ij")
    u = xs.reshape(-1)
    v = ys.reshape(-1)

    in_maps = []
    per_batch = []
    for b in range(B):
        fx, fy, x0, y0 = pix[b, 0, 0], pix[b, 1, 1], pix[b, 0, 2], pix[b, 1, 2]
        z = _q16(dep[b])
        X = _q16((u - x0) * dep[b] / fx)
        Y = _q16((v - y0) * dep[b] / fy)
        T0 = tm[b].reshape(N, 4, 4)
        dR = _q16(T0[:, :3, :3] - np.eye(3))   # rotations are near identity
        R = np.eye(3) + dR
        t = _q16(T0[:, :3, 3])
        xyz = np.stack([X, Y, z], -1)
        TjXj = np.einsum("kpq,kq->kp", R, xyz) + t
        Xkk = TjXj[:, 0] / TjXj[:, 2]
        Ykk = TjXj[:, 1] / TjXj[:, 2]
        dkk = 1.0 / TjXj[:, 2]
        on, zn = np.ones(N), np.zeros(N)
        JT0 = np.stack([on, zn, zn, zn, -z, Y], -1)
        JT1 = np.stack([zn, on, zn, z, zn, -X], -1)
        JT2 = np.stack([zn, zn, on, -Y, X, zn], -1)
        G0 = JT0 - Xkk[:, None] * JT2
        G1 = JT1 - Ykk[:, None] * JT2
        w0, w1, w2 = wgt[b, 0], wgt[b, 1], wgt[b, 2]
        r0, r1, r2 = rev[b, 0], rev[b, 1], rev[b, 2]

        def outer(a, bb):
            return np.einsum("kp,kq->kpq", a, bb)

        P00 = outer(G0, G0)
        P11 = outer(G1, G1)
        P22 = outer(JT2, JT2)
        P02 = outer(G0, JT2) + outer(JT2, G0)
        P12 = outer(G1, JT2) + outer(JT2, G1)
        wfx = (w0 * fx * fx)[:, None, None]
        wfy = (w1 * fy * fy)[:, None, None]
        SH = [wfx * P00 + wfy * P11, -wfx * P02, -wfy * P12,
              wfx * P22, wfy * P22, w2[:, None, None] * P22]
        SR = [-(fx * r0)[:, None] * G0 - (fy * r1)[:, None] * G1,
              r2[:, None] * JT2,
              fx * fx * G0 + (fx * r0)[:, None] * JT2,
              fy * fy * G1 + (fy * r1)[:, None] * JT2,
              -fx * fx * JT2,
              -fy * fy * JT2,
              -JT2]
        lam = 1.0 / (fx * fx)
        # acc stationary [128, KC*ACC_CW]; partition p of chunk c is k=c*128+p
        accst = np.zeros((P, KC * ACC_CW), f6)
        for c in range(KC):
            ks = slice(c * P, (c + 1) * P)
            co = c * ACC_CW
            for m, S in enumerate(SH):
                for ei, (p_, q_) in enumerate(HTRI):
                    accst[:, co + m * 21 + ei] = S[ks, p_, q_] * lam
            ro = co + NHB * 21
            for m, V in enumerate(SR):
                accst[:, ro + m * 6 : ro + (m + 1) * 6] = V[ks] * lam
        # geometry stationary [32, N]
        stat = np.zeros((GR, N), f6)
        stat[0:16] = -2.0 * emb[b]
        stat[16], stat[17], stat[18], stat[19] = X, Y, z, 1.0
        stat[20:24] = _q16(Xkk[None] * stat[16:20])
        stat[24:28] = _q16(Ykk[None] * stat[16:20])
        stat[28] = _q16((emb[b] ** 2).sum(0))
        stat[29] = _q16(X - Xkk * z)
        stat[30] = _q16(Y - Ykk * z)
        stat[31] = z
        per_batch.append(dict(stat=stat, accst=accst, dkk=dkk,
                              emb=emb[b], dR=dR, t=t))

    cmbH = np.zeros((P, 48), np.float32)
    for ei, (p_, q_) in enumerate(HTRI):
        cmbH[ei, p_ * 7 + q_] = 1.0
        if p_ != q_:
            cmbH[ei, q_ * 7 + p_] = 1.0
    cmbR = np.zeros((P, 48), np.float32)
    for p_ in range(6):
        cmbR[p_, p_ * 7 + 6] = 1.0

    for core in range(NCORES):
        b = core // 4
        s0 = (core % 4) * SLAB
        pb = per_batch[b]
        dRs = pb["dR"][s0 : s0 + SLAB]
        ts = pb["t"][s0 : s0 + SLAB]
        # moving operand [32, 4*SLAB]: X' | Y' | Z | s blocks
        mov = np.zeros((GR, 4 * SLAB), f6)
        for blk, row in ((0, 0), (1, 1), (2, 2)):
            mov[16:19, blk * SLAB : (blk + 1) * SLAB] = dRs[:, row, :].T
            mov[19, blk * SLAB : (blk + 1) * SLAB] = ts[:, row]
        mov[20:23, 0:SLAB] = -dRs[:, 2, :].T
        mov[23, 0:SLAB] = -ts[:, 2]
        mov[24:27, SLAB : 2 * SLAB] = -dRs[:, 2, :].T
        mov[27, SLAB : 2 * SLAB] = -ts[:, 2]
        mov[29, 0:SLAB] = 1.0
        mov[30, SLAB : 2 * SLAB] = 1.0
        mov[31, 2 * SLAB : 3 * SLAB] = 1.0
        ei_ = pb["emb"][:, s0 : s0 + SLAB]
        mov[0:16, 3 * SLAB : 4 * SLAB] = ei_
        mov[19, 3 * SLAB : 4 * SLAB] = _q16((ei_ ** 2).sum(0))
        mov[28, 3 * SLAB : 4 * SLAB] = 1.0

        geom = np.concatenate([pb["stat"], mov], 1)

        misc = np.zeros((P, 136), np.float32)
        misc[:, 0:8] = pb["dkk"].reshape(KC, P).T
        tms = np.asarray(tm[b][s0 : s0 + SLAB], np.float32)
        misc[:, 8:24] = tms[0:P]
        misc[:, 24:40] = tms[P : 2 * P]
        misc[:, 40:88] = cmbH
        misc[:, 88:136] = cmbR

        in_maps.append({
            "geom": np.ascontiguousarray(geom, np.float16),
            "accst": np.ascontiguousarray(pb["accst"], np.float16),
            "misc": np.ascontiguousarray(misc),
        })
    return in_maps


def gather_output(results):
    full = np.empty((B, N, 16), dtype=np.float32)
    for core in range(NCORES):
        b = core // 4
        s0 = (core % 4) * SLAB
        out = results[core]["out"]
        full[b, s0 : s0 + P] = out[:, 0:16]
        full[b, s0 + P : s0 + SLAB] = out[:, 16:32]
    return full.reshape(B, H, W, 4, 4)


_NC_CACHE = {}


def kernel(**inputs):
    if "nc" not in _NC_CACHE:
        _NC_CACHE["nc"] = build_nc()
    nc = _NC_CACHE["nc"]
    in_maps = prep_inputs(**inputs)
    res = run_bass_kernel_spmd(nc, in_maps, core_ids=list(range(NCORES)))
    return gather_output(res.results)
